# revision 1
# baseline (speedup 1.0000x reference)
"""GraphSAGE (2-layer, mean-agg) Trainium2 Bass kernel, 8-core SPMD.

Design: shard dst nodes across 8 cores (6250 each). Edges partitioned by dst
owner, sorted by dst, grouped into 128-dst windows. Messages fetched with
gpsimd dma_gather (bf16 tables, single_packet=False); segment-sum done on the
PE via per-rank selection-matrix matmuls accumulated in PSUM per window
(scatter-add CCE has a duplicate-index race on HW, so no scatters are used).
Layer-2 aggregates p = h @ w2_l (40->128-col padded bf16) instead of h
(512-dim): p shards are AllGathered in two 3125-row slices so gather indices
fit int16.
"""
import numpy as np
import ml_dtypes

N = 50000
E = 800000
DIN, HID, OUT = 128, 512, 40
NCORES = 8
NLOC = N // NCORES          # 6250
P = 128
NWIN = (NLOC + P - 1) // P  # 49
NPAD = NWIN * P             # 6272
XSPLIT = 32768              # x table split for int16 gather indices
SLICE_LEN = 1568            # p-table allgather slice length (4 slices)
NSLICE = 4
L1_CHUNK = 2                # windows per L1 gather call group
L2_CHUNK = 4

bf16 = ml_dtypes.bfloat16


def _build_schedule(edge_index):
    """Per-core, per-layer edge orderings + the cross-core-common rank schedule."""
    src = np.asarray(edge_index[0], dtype=np.int64)
    dst = np.asarray(edge_index[1], dtype=np.int64)
    deg = np.bincount(dst, minlength=N).astype(np.float32)
    recip = 1.0 / np.maximum(deg, 1.0)

    per_core = []
    for c in range(NCORES):
        lo, hi = c * NLOC, (c + 1) * NLOC
        m = (dst >= lo) & (dst < hi)
        s, d = src[m], dst[m] - lo
        per_core.append((s, d))

    # bucket key per layer: L1 by src>=XSPLIT, L2 by (src % NLOC) // PSLICE
    def buckets(s):
        return [s >= XSPLIT, (s % NLOC) // SLICE_LEN]

    # counts[layer][core][win][bucket]
    counts = np.zeros((2, NCORES, NWIN, 4), np.int64)
    percore_lists = []  # [core][layer][win][bucket] -> (gidx array, drel array)
    for c in range(NCORES):
        s, d = per_core[c]
        w = d // P
        bk = buckets(s)
        layers = []
        for L in range(2):
            nb = 2 if L == 0 else NSLICE
            b = bk[L].astype(np.int64)
            order = np.lexsort((b, w))  # by window, then bucket
            ss, dd, ww, bb = s[order], d[order], w[order], b[order]
            wins = []
            for wi in range(NWIN):
                sel = ww == wi
                ssw, ddw, bbw = ss[sel], dd[sel], bb[sel]
                ents = []
                for bu in range(nb):
                    q = bbw == bu
                    sq, dq = ssw[q], ddw[q]
                    if L == 0:
                        gi = np.where(sq >= XSPLIT, sq - XSPLIT, sq)
                    else:
                        u = sq % NLOC
                        gi = (sq // NLOC) * SLICE_LEN + (u - (u // SLICE_LEN) * SLICE_LEN)
                    counts[L, c, wi, bu] = len(sq)
                    ents.append((gi.astype(np.int64), (dq - wi * P).astype(np.int64)))
                wins.append(ents)
            layers.append(wins)
        percore_lists.append(layers)

    # common rank schedule: ranks[L][win][bucket] = ceil(max_c count /128), >=0
    ranks = np.zeros((2, NWIN, 4), np.int64)
    for L in range(2):
        mx = counts[L].max(axis=0)  # [NWIN, 2]
        ranks[L] = (mx + P - 1) // P
        for wi in range(NWIN):  # ensure every window has >=1 rank total
            if ranks[L, wi].sum() == 0:
                ranks[L, wi, 0] = 1
    return per_core, percore_lists, ranks, recip


def _wrap_call(flat_idx):
    """int16 wrapped layout for one gather call: slot i -> [i%16, i//16]."""
    n = len(flat_idx)
    w = flat_idx.astype(np.int16).reshape(n // 16, 16).T.copy()
    return np.tile(w, (8, 1))  # [128, n/16]


def _pack_layer(layers_for_core, ranks, L, chunk):
    """Build idx [128, T16] int16, drel [128, R] bf16 and call/window metadata.
    Call layout per chunk: [bucket0: win a..b segs][bucket1: win a..b segs]."""
    idx_cols, drel_cols = [], []
    calls = []      # (bucket, rank_off, n_ranks)
    win_ranges = [] # per window: list of (rank_start, rank_end)
    rank_off = 0
    for c0 in range(0, NWIN, chunk):
        cw = range(c0, min(c0 + chunk, NWIN))
        for bu in range(2 if L == 0 else NSLICE):
            seg_ranks = int(sum(ranks[L, wi, bu] for wi in cw))
            if seg_ranks == 0:
                continue
            flat = np.zeros(seg_ranks * P, np.int64)
            drel = np.full(seg_ranks * P, -1, np.int64)
            off = 0
            for wi in cw:
                nr = int(ranks[L, wi, bu])
                if nr == 0:
                    continue
                gi, dq = layers_for_core[wi][bu]
                flat[off:off + len(gi)] = gi
                drel[off:off + len(dq)] = dq
                if len(win_ranges) <= wi - 0:
                    pass
                win_ranges.append((wi, rank_off + off // P, rank_off + off // P + nr))
                off += nr * P
            idx_cols.append(_wrap_call(flat))
            # drel slot i -> partition i%128, rank i//128
            drel_cols.append(drel.reshape(seg_ranks, P).T.astype(bf16))
            calls.append((bu, rank_off, seg_ranks))
            rank_off += seg_ranks
    idx_arr = np.concatenate(idx_cols, axis=1)
    drel_arr = np.concatenate(drel_cols, axis=1)
    # merge win_ranges into per-window lists
    wmap = [[] for _ in range(NWIN)]
    for wi, a, b in win_ranges:
        wmap[wi].append((a, b))
    return idx_arr, drel_arr, calls, wmap


def kernel(x, edge_index, w1_l, b1, w1_r, w2_l, b2, w2_r):
    import concourse.bacc as bacc
    import concourse.bass as bass
    import concourse.mybir as mybir
    import concourse.tile as tile
    from concourse.bass_utils import run_bass_kernel_spmd
    from concourse.library_config import mlp
    from concourse.masks import make_identity

    x = np.asarray(x, np.float32)
    per_core, percore_lists, ranks, recip = _build_schedule(np.asarray(edge_index))

    # ---- host-side packed arrays (same shapes on every core) ----
    core_arrays = []
    for c in range(NCORES):
        i1, d1, calls1, wmap1 = _pack_layer(percore_lists[c][0], ranks, 0, L1_CHUNK)
        i2, d2, calls2, wmap2 = _pack_layer(percore_lists[c][1], ranks, 1, L2_CHUNK)
        core_arrays.append((i1, d1, i2, d2))
    calls1, wmap1, calls2, wmap2 = calls1, wmap1, calls2, wmap2  # same all cores

    xlo = np.zeros((XSPLIT, DIN), bf16); xlo[:] = x[:XSPLIT].astype(bf16)
    xhi = np.zeros((N - XSPLIT, DIN), bf16); xhi[:] = x[XSPLIT:].astype(bf16)
    iota_np = np.tile(np.arange(P, dtype=np.float32), (P, 1)).astype(bf16)
    b2b_np = np.tile(np.asarray(b2, np.float32)[None, :], (P, 1))

    T16_1, R1 = core_arrays[0][0].shape[1], core_arrays[0][1].shape[1]
    T16_2, R2 = core_arrays[0][2].shape[1], core_arrays[0][3].shape[1]

    nc = bacc.Bacc("TRN2")
    dt = mybir.dt
    t_xlo = nc.declare_dram_parameter("xlo", [XSPLIT, DIN], dt.bfloat16, isOutput=False)
    t_xhi = nc.declare_dram_parameter("xhi", [N - XSPLIT, DIN], dt.bfloat16, isOutput=False)
    t_xoT = nc.declare_dram_parameter("xoT", [P, NPAD], dt.bfloat16, isOutput=False)
    t_i1 = nc.declare_dram_parameter("i1", [P, T16_1], dt.int16, isOutput=False)
    t_d1 = nc.declare_dram_parameter("d1", [P, R1], dt.bfloat16, isOutput=False)
    t_i2 = nc.declare_dram_parameter("i2", [P, T16_2], dt.int16, isOutput=False)
    t_d2 = nc.declare_dram_parameter("d2", [P, R2], dt.bfloat16, isOutput=False)
    t_w1l = nc.declare_dram_parameter("w1l", [DIN, HID], dt.bfloat16, isOutput=False)
    t_w1r = nc.declare_dram_parameter("w1r", [DIN, HID], dt.bfloat16, isOutput=False)
    t_w2l = nc.declare_dram_parameter("w2l", [P, HID // P, OUT], dt.bfloat16, isOutput=False)
    t_w2r = nc.declare_dram_parameter("w2r", [P, HID // P, OUT], dt.bfloat16, isOutput=False)
    t_b1 = nc.declare_dram_parameter("b1", [P, HID // P], dt.float32, isOutput=False)
    t_b2 = nc.declare_dram_parameter("b2b", [P, OUT], dt.float32, isOutput=False)
    t_rc = nc.declare_dram_parameter("rc", [P, NWIN], dt.float32, isOutput=False)
    t_iota = nc.declare_dram_parameter("iota", [P, P], dt.bfloat16, isOutput=False)
    t_iota4 = nc.declare_dram_parameter("iota4", [P, 4, P], dt.bfloat16, isOutput=False)
    t_out = nc.declare_dram_parameter("out", [NPAD, OUT], dt.float32, isOutput=True)

    pS = [nc.dram_tensor(f"p{s}", [SLICE_LEN, P], dt.bfloat16) for s in range(NSLICE)]
    pgS = [nc.dram_tensor(f"pg{s}", [NCORES * SLICE_LEN, P], dt.bfloat16, addr_space="Shared") for s in range(NSLICE)]

    AluOp = mybir.AluOpType
    AF = mybir.ActivationFunctionType

    with tile.TileContext(nc) as tc:
        with tc.tile_pool(name="const", bufs=1) as cpool, \
             tc.tile_pool(name="msg", bufs=2) as mpool, \
             tc.tile_pool(name="sm", bufs=3) as spool, \
             tc.tile_pool(name="work", bufs=3) as wpool, \
             tc.tile_pool(name="psumA", bufs=2, space="PSUM") as ppool, \
             tc.tile_pool(name="psumB", bufs=1, space="PSUM") as ppoolb:
            nc.gpsimd.load_library(mlp)
            ident = cpool.tile([P, P], dt.bfloat16)
            make_identity(nc, ident[:])
            iota_t = cpool.tile([P, P], dt.bfloat16)
            nc.sync.dma_start(iota_t[:], t_iota[:])
            iota4_t = cpool.tile([P, 4, P], dt.bfloat16)
            nc.sync.dma_start(iota4_t[:], t_iota4[:])
            i1_t = cpool.tile([P, T16_1], dt.int16)
            nc.sync.dma_start(i1_t[:], t_i1[:])
            d1_t = cpool.tile([P, R1], dt.bfloat16)
            nc.sync.dma_start(d1_t[:], t_d1[:])
            i2_t = cpool.tile([P, T16_2], dt.int16)
            nc.sync.dma_start(i2_t[:], t_i2[:])
            d2_t = cpool.tile([P, R2], dt.bfloat16)
            nc.sync.dma_start(d2_t[:], t_d2[:])
            xoT_t = cpool.tile([P, NPAD], dt.bfloat16)
            nc.sync.dma_start(xoT_t[:], t_xoT[:])
            w1l_t = cpool.tile([DIN, HID], dt.bfloat16)
            nc.sync.dma_start(w1l_t[:], t_w1l[:])
            w1r_t = cpool.tile([DIN, HID], dt.bfloat16)
            nc.sync.dma_start(w1r_t[:], t_w1r[:])
            w2l_t = cpool.tile([P, HID // P, OUT], dt.bfloat16)
            nc.sync.dma_start(w2l_t[:], t_w2l[:])
            w2r_t = cpool.tile([P, HID // P, OUT], dt.bfloat16)
            nc.sync.dma_start(w2r_t[:], t_w2r[:])
            b1_t = cpool.tile([P, HID // P], dt.float32)
            nc.sync.dma_start(b1_t[:], t_b1[:])
            b2_t = cpool.tile([P, OUT], dt.float32)
            nc.sync.dma_start(b2_t[:], t_b2[:])
            rc_t = cpool.tile([P, NWIN], dt.float32)
            nc.sync.dma_start(rc_t[:], t_rc[:])
            qbuf = cpool.tile([P, NWIN, OUT], dt.float32)

            # ---------- Layer 1 + stage B, chunked ----------
            call_i = 0
            cum16 = 0
            for c0 in range(0, NWIN, L1_CHUNK):
                cw = list(range(c0, min(c0 + L1_CHUNK, NWIN)))
                chunk_ranks = int(sum(ranks[0, wi, :].sum() for wi in cw))
                if chunk_ranks == 0:
                    continue
                msg = mpool.tile([P, chunk_ranks, DIN], dt.bfloat16, tag="msg1")
                base_rank = None
                # issue this chunk's gather calls
                local_off = 0
                while call_i < len(calls1):
                    bu, roff, nr = calls1[call_i]
                    # does this call belong to the current chunk?
                    if base_rank is None:
                        base_rank = roff
                    if roff - base_rank >= chunk_ranks:
                        break
                    n_idx = nr * P
                    tblap = t_xlo[:] if bu == 0 else t_xhi[:]
                    nc.gpsimd.dma_gather(
                        msg[:, roff - base_rank:roff - base_rank + nr, :],
                        tblap, i1_t[:, cum16:cum16 + n_idx // 16],
                        n_idx, n_idx, DIN, single_packet=False)
                    cum16 += n_idx // 16
                    local_off += nr
                    call_i += 1
                # per-window segmented reduction + stage B
                for wi in cw:
                    segs = [(a - base_rank, b - base_rank) for a, b in wmap1[wi]]
                    nseg = sum(b - a for a, b in segs)
                    pagg = ppool.tile([P, P], dt.float32, tag="pagg")
                    first = True
                    for a, b in segs:
                        r = a
                        while r < b:
                            kk = min(4, b - r)
                            S = spool.tile([P, 4, P], dt.bfloat16, tag="S1")
                            nc.vector.tensor_tensor(
                                out=S[:, :kk, :],
                                in0=d1_t[:, base_rank + r:base_rank + r + kk, None].to_broadcast([P, kk, P]),
                                in1=iota4_t[:, :kk, :], op=AluOp.is_equal)
                            for j in range(kk):
                                nc.tensor.matmul(pagg[:], lhsT=S[:, j, :], rhs=msg[:, r + j, :],
                                                 start=first, stop=(r + j == b - 1 and (a, b) == segs[-1]))
                                first = False
                            r += kk
                    am = wpool.tile([P, DIN], dt.bfloat16, tag="am")
                    nc.scalar.activation(am[:], pagg[:], AF.Copy, scale=rc_t[:, wi:wi + 1])
                    pamT = ppoolb.tile([P, P], dt.bfloat16, tag="pamT")
                    nc.tensor.transpose(out=pamT[:], in_=am[:], identity=ident[:])
                    amT = wpool.tile([P, P], dt.bfloat16, tag="amT")
                    nc.scalar.activation(amT[:], pamT[:], AF.Copy)
                    # h blocks + p/q
                    pq = ppool.tile([P, OUT], dt.float32, tag="pq")
                    qq = ppool.tile([P, OUT], dt.float32, tag="qq")
                    for bjj in range(HID // P):
                        ph = ppoolb.tile([P, P], dt.float32, tag="ph")
                        nc.tensor.matmul(ph[:], lhsT=w1l_t[:, bjj * P:(bjj + 1) * P], rhs=amT[:], start=True, stop=False)
                        nc.tensor.matmul(ph[:], lhsT=w1r_t[:, bjj * P:(bjj + 1) * P], rhs=xoT_t[:, wi * P:(wi + 1) * P], start=False, stop=True)
                        hT = wpool.tile([P, P], dt.bfloat16, tag="hT")
                        nc.scalar.activation(hT[:], ph[:], AF.Relu, bias=b1_t[:, bjj:bjj + 1])
                        nc.tensor.matmul(pq[:], lhsT=hT[:], rhs=w2l_t[:, bjj, :], start=(bjj == 0), stop=(bjj == 3))
                        nc.tensor.matmul(qq[:], lhsT=hT[:], rhs=w2r_t[:, bjj, :], start=(bjj == 0), stop=(bjj == 3))
                    nc.scalar.activation(qbuf[:, wi, :], qq[:], AF.Copy)
                    pt = wpool.tile([P, P], dt.bfloat16, tag="pt")
                    nc.vector.memset(pt[:], 0.0)
                    nc.scalar.activation(pt[:, :OUT], pq[:], AF.Copy)
                    r0, r1_ = wi * P, min((wi + 1) * P, NLOC)
                    for s in range(NSLICE):
                        a0, a1 = s * SLICE_LEN, min((s + 1) * SLICE_LEN, NLOC)
                        c0_, c1_ = max(r0, a0), min(r1_, a1)
                        if c0_ < c1_:
                            nc.sync.dma_start(pS[s][c0_ - a0:c1_ - a0, :], pt[c0_ - r0:c1_ - r0, :])

            # ---------- AllGather p slices ----------
            for s in range(NSLICE):
                nc.gpsimd.collective_compute(
                    "AllGather", AluOp.bypass, replica_groups=[list(range(NCORES))],
                    ins=[pS[s][:]], outs=[pgS[s][:]])

            # ---------- Layer 2 + output ----------
            call_i = 0
            cum16 = 0
            for c0 in range(0, NWIN, L2_CHUNK):
                cw = list(range(c0, min(c0 + L2_CHUNK, NWIN)))
                chunk_ranks = int(sum(ranks[1, wi, :].sum() for wi in cw))
                if chunk_ranks == 0:
                    continue
                msg = mpool.tile([P, chunk_ranks, P], dt.bfloat16, tag="msg2")
                base_rank = None
                while call_i < len(calls2):
                    bu, roff, nr = calls2[call_i]
                    if base_rank is None:
                        base_rank = roff
                    if roff - base_rank >= chunk_ranks:
                        break
                    n_idx = nr * P
                    tblap = pgS[bu][:]
                    nc.gpsimd.dma_gather(
                        msg[:, roff - base_rank:roff - base_rank + nr, :],
                        tblap, i2_t[:, cum16:cum16 + n_idx // 16],
                        n_idx, n_idx, P, single_packet=False)
                    cum16 += n_idx // 16
                    call_i += 1
                for wi in cw:
                    segs = [(a - base_rank, b - base_rank) for a, b in wmap2[wi]]
                    pagg = ppool.tile([P, P], dt.float32, tag="pagg")
                    first = True
                    for a, b in segs:
                        r = a
                        while r < b:
                            kk = min(4, b - r)
                            S = spool.tile([P, 4, P], dt.bfloat16, tag="S2")
                            nc.vector.tensor_tensor(
                                out=S[:, :kk, :],
                                in0=d2_t[:, base_rank + r:base_rank + r + kk, None].to_broadcast([P, kk, P]),
                                in1=iota4_t[:, :kk, :], op=AluOp.is_equal)
                            for j in range(kk):
                                nc.tensor.matmul(pagg[:], lhsT=S[:, j, :], rhs=msg[:, r + j, :],
                                                 start=first, stop=(r + j == b - 1 and (a, b) == segs[-1]))
                                first = False
                            r += kk
                    z = wpool.tile([P, OUT], dt.float32, tag="z")
                    nc.vector.tensor_tensor(out=z[:], in0=pagg[:, :OUT],
                                            in1=rc_t[:, wi:wi + 1].to_broadcast([P, OUT]),
                                            op=AluOp.mult)
                    nc.vector.tensor_tensor(out=z[:], in0=z[:], in1=qbuf[:, wi, :], op=AluOp.add)
                    nc.vector.tensor_tensor(out=z[:], in0=z[:], in1=b2_t[:], op=AluOp.add)
                    mneg = wpool.tile([P, 1], dt.float32, tag="mneg")
                    nc.vector.tensor_reduce(mneg[:], z[:], axis=mybir.AxisListType.X, op=AluOp.max, negate=True)
                    ez = wpool.tile([P, OUT], dt.float32, tag="ez")
                    nc.scalar.activation(ez[:], z[:], AF.Exp, bias=mneg[:])
                    ssum = wpool.tile([P, 1], dt.float32, tag="ssum")
                    nc.vector.tensor_reduce(ssum[:], ez[:], axis=mybir.AxisListType.X, op=AluOp.add)
                    lsum = wpool.tile([P, 1], dt.float32, tag="lsum")
                    nc.scalar.activation(lsum[:], ssum[:], AF.Ln)
                    nc.vector.tensor_tensor(out=lsum[:], in0=lsum[:], in1=mneg[:], op=AluOp.subtract)
                    ot = wpool.tile([P, OUT], dt.float32, tag="ot")
                    nc.vector.tensor_tensor(out=ot[:], in0=z[:], in1=lsum[:].to_broadcast([P, OUT]), op=AluOp.subtract)
                    nc.sync.dma_start(t_out[wi * P:(wi + 1) * P, :], ot[:])

    nc.compile()

    in_maps = []
    for c in range(NCORES):
        i1a, d1a, i2a, d2a = core_arrays[c]
        xoT = np.zeros((P, NPAD), bf16)
        xoT[:, :NLOC] = x[c * NLOC:(c + 1) * NLOC].T.astype(bf16)
        rcf = np.ones(NPAD, np.float32)
        rcf[:NLOC] = recip[c * NLOC:(c + 1) * NLOC]
        rcc = rcf.reshape(NWIN, P).T.copy()
        in_maps.append({
            "xlo": xlo, "xhi": xhi, "xoT": xoT,
            "i1": i1a, "d1": d1a, "i2": i2a, "d2": d2a,
            "w1l": np.asarray(w1_l).astype(bf16), "w1r": np.asarray(w1_r).astype(bf16),
            "w2l": np.ascontiguousarray(np.asarray(w2_l).astype(bf16).reshape(HID // P, P, OUT).transpose(1, 0, 2)), "w2r": np.ascontiguousarray(np.asarray(w2_r).astype(bf16).reshape(HID // P, P, OUT).transpose(1, 0, 2)),
            "b1": np.asarray(b1, np.float32).reshape(HID // P, P).T.copy(),
            "b2b": b2b_np, "rc": rcc,
            "iota": iota_np, "iota4": np.ascontiguousarray(np.broadcast_to(iota_np[:, None, :], (128, 4, 128))),
        })
    res = run_bass_kernel_spmd(nc, in_maps, list(range(NCORES)))
    out = np.concatenate([res.results[c]["out"][:NLOC] for c in range(NCORES)], axis=0)
    kernel.last_results = res
    kernel.last_nc = nc
    return out.astype(np.float32)



# revision 43
# speedup vs baseline: 1.8737x; 1.8737x over previous
"""GraphSAGE (2-layer, mean-agg) Trainium2 Bass kernel, 8-core SPMD.

Layer 1: dst-sharded (6250 nodes/core, 49 windows of 128). Messages x[src]
fetched with gpsimd dma_gather (bf16 tables split at 32768 so indices fit
int16); segment-sum via per-rank selection-matrix matmuls accumulated in PSUM.
Window slots are 16-granular (not 128) inside each chunk call; chunk-relative
int16 drel + per-window iota tables disambiguate shared boundary ranks.

Layer 2: src-sharded. Each core computes p = relu(h) @ w2_l for its own nodes,
writes it to a private DRAM table, gathers its own-src edges' p rows (single
int16 bucket), and accumulates partial dst sums for all 8 slabs of 49 windows.
One bf16 ReduceScatter (4MB -> 0.5MB) replaces the baseline's 4 AllGathers.

Output: z = rs*recip + (h@w2_r + b2); log_softmax computed with batched Exp
over all windows and a single Ln (avoids activation-table reloads); logits
are within +-5 so no max-subtraction is needed.
"""
import numpy as np
import ml_dtypes

N = 50000
E = 800000
DIN, HID, OUT = 128, 512, 40
NCORES = 8
NLOC = N // NCORES          # 6250
P = 128
NWIN = (NLOC + P - 1) // P  # 49
NPAD = NWIN * P             # 6272
XSPLIT = 32768              # x table split for int16 gather indices
CW1 = 5                     # L1 windows per gather chunk
CW2 = 25                    # L2 windows per gather chunk (within a slab)
KB = 4                      # selection-matrix ranks per DVE build op

bf16 = ml_dtypes.bfloat16


def _cdiv(a, b):
    return -(-a // b)


def _wrap_idx(flat):
    """int16 wrapped layout for one gather call: slot i -> [i%16, i//16]."""
    n = len(flat)
    w = flat.astype(np.int16).reshape(n // 16, 16).T.copy()
    return np.tile(w, (8, 1))  # [128, n/16]


def _chunk_ranges(nwin_total, cw, period):
    """Window ranges of <=cw windows that never cross a period boundary."""
    out = []
    for p0 in range(0, nwin_total, period):
        pend = min(p0 + period, nwin_total)
        for w0 in range(p0, pend, cw):
            out.append((w0, min(w0 + cw, pend)))
    return out


def _build_layer(edge_core_lists, nwin_total, cw, nbuckets, period=None,
                 edge_scale=None):
    """Common (cross-core) schedule for one layer.

    edge_core_lists: per core, (idx, drel_global, win, bucket) arrays where
    win in [0, nwin_total), drel_global = dst offset within the window space
    (win*128 + in-window row). edge_scale: per core, per-edge scale values
    packed alongside drel (selection-matrix entries become this scale).
    Returns (per-core packed (idx16, d, scale) arrays, chunk descriptors,
    totals).
    """
    counts = np.zeros((NCORES, nwin_total, nbuckets), np.int64)
    for c, (gi, dg, win, bk) in enumerate(edge_core_lists):
        np.add.at(counts[c], (win, bk), 1)
    seg16 = (_cdiv_arr(counts.max(axis=0), 16) * 16).astype(np.int64)  # [nwin, nb]
    # a 128-slot rank must never span 3 windows: with >=128-slot segments a
    # rank touches only adjacent windows, which the parity offset in drel
    # disambiguates (values stay < 256 so they are exact in bf16)
    seg16[seg16 > 0] = np.maximum(seg16[seg16 > 0], P)

    chunks = []
    d_off = 0
    i16_off = 0
    for w0, wend in _chunk_ranges(nwin_total, cw, period or nwin_total):
        ws = list(range(w0, wend))
        calls = []
        windows = {w: [] for w in ws}
        msg_off = 0
        for b in range(nbuckets):
            call_len = int(seg16[ws, b].sum())
            if call_len == 0:
                continue
            n_ranks = _cdiv(call_len, P)
            off = 0
            for w in ws:
                sl = int(seg16[w, b])
                if sl == 0:
                    continue
                ra, rb = off >> 7, _cdiv(off + sl, P)
                windows[w].append((b, d_off + ra, msg_off + ra, rb - ra))
                off += sl
            calls.append((b, i16_off, call_len, msg_off, n_ranks, d_off))
            d_off += n_ranks
            i16_off += call_len // 16
            msg_off += n_ranks
        chunks.append({
            "w0": w0, "ws": ws, "calls": calls, "windows": windows,
            "n_ranks": msg_off,
        })

    # per-core packed arrays
    per_core = []
    for c, (gi, dg, win, bk) in enumerate(edge_core_lists):
        i16_cols = np.zeros((P, i16_off), np.int16)
        d_cols = np.full((P, d_off), -1, np.float32)
        s_cols = np.zeros((P, d_off), np.float32)
        sv = edge_scale[c] if edge_scale is not None else None
        # bucket edge data sorted by (win, bucket) for slot placement
        order = np.lexsort((bk, win))
        gi_s, dg_s, win_s, bk_s = gi[order], dg[order], win[order], bk[order]
        sv_s = sv[order] if sv is not None else None
        # start index of each (win,bucket) group in sorted arrays
        key = win_s * nbuckets + bk_s
        starts = np.searchsorted(key, np.arange(nwin_total * nbuckets))
        ends = np.searchsorted(key, np.arange(nwin_total * nbuckets), side="right")
        for ch in chunks:
            for (b, i16o, call_len, mo, n_ranks, do) in ch["calls"]:
                flat = np.zeros(n_ranks * P, np.int64)
                drel = np.full(n_ranks * P, -1, np.int64)
                sval = np.zeros(n_ranks * P, np.float32)
                off = 0
                for w in ch["ws"]:
                    sl = int(seg16[w, b])
                    if sl == 0:
                        continue
                    a, e = starts[w * nbuckets + b], ends[w * nbuckets + b]
                    cnt = e - a
                    flat[off:off + cnt] = gi_s[a:e]
                    # window-relative row + parity offset (exact in bf16)
                    drel[off:off + cnt] = (dg_s[a:e] - w * P) + (w & 1) * P
                    if sv_s is not None:
                        sval[off:off + cnt] = sv_s[a:e]
                    off += sl
                i16_cols[:, i16o:i16o + call_len // 16] = _wrap_idx(flat[:call_len])
                d_cols[:, do:do + n_ranks] = drel.reshape(n_ranks, P).T.astype(np.float32)
                s_cols[:, do:do + n_ranks] = sval.reshape(n_ranks, P).T
        per_core.append((i16_cols, d_cols, s_cols))
    return per_core, chunks, i16_off, d_off


def _cdiv_arr(a, b):
    return -(-a // b)


def _build_schedule(edge_index):
    src = np.asarray(edge_index[0], dtype=np.int64)
    dst = np.asarray(edge_index[1], dtype=np.int64)
    deg = np.bincount(dst, minlength=N).astype(np.float32)
    recip = 1.0 / np.maximum(deg, 1.0)

    # ---- L1: dst-sharded; selection entries carry 1/deg so the PSUM sum
    # is already the mean ----
    l1_lists, l1_scales = [], []
    for c in range(NCORES):
        lo, hi = c * NLOC, (c + 1) * NLOC
        m = (dst >= lo) & (dst < hi)
        s, dg = src[m], dst[m] - lo
        bk = (s >= XSPLIT).astype(np.int64)
        gi = np.where(bk == 1, s - XSPLIT, s)
        l1_lists.append((gi, dg, dg >> 7, bk))
        l1_scales.append(recip[dst[m]])
    l1_pc, l1_chunks, T16_1, R1 = _build_layer(l1_lists, NWIN, CW1, 2,
                                               edge_scale=l1_scales)

    # ---- L2: src-sharded, windows = slab*NWIN + within-slab window ----
    l2_lists = []
    for c in range(NCORES):
        lo, hi = c * NLOC, (c + 1) * NLOC
        m = (src >= lo) & (src < hi)
        s, d = src[m] - lo, dst[m]
        slab = d // NLOC
        rel = d - slab * NLOC
        win = slab * NWIN + (rel >> 7)
        dg = win * P + (rel & (P - 1))
        l2_lists.append((s, dg, win, np.zeros(len(s), np.int64)))
    l2_pc, l2_chunks, T16_2, R2 = _build_layer(l2_lists, NWIN * NCORES, CW2, 1,
                                               period=NWIN)

    return l1_pc, l1_chunks, T16_1, R1, l2_pc, l2_chunks, T16_2, R2, recip


def kernel(x, edge_index, w1_l, b1, w1_r, w2_l, b2, w2_r):
    import concourse.bacc as bacc
    import concourse.mybir as mybir
    import concourse.tile as tile
    from concourse.bass_utils import run_bass_kernel_spmd
    from concourse.library_config import mlp
    from concourse.masks import make_identity

    x = np.asarray(x, np.float32)
    (l1_pc, l1_chunks, T16_1, R1,
     l2_pc, l2_chunks, T16_2, R2, recip) = _build_schedule(np.asarray(edge_index))

    MR1 = max(ch["n_ranks"] for ch in l1_chunks)   # msg tile ranks (L1)
    MR2 = max(ch["n_ranks"] for ch in l2_chunks)
    MR = max(MR1, MR2)

    xlo = np.ascontiguousarray(x[:XSPLIT].astype(bf16))
    xhi = np.ascontiguousarray(x[XSPLIT:].astype(bf16))
    b2b_np = np.tile(np.asarray(b2, np.float32)[None, :], (P, 1))
    iota = np.arange(P, dtype=np.int64)
    iopar_np = np.broadcast_to((iota[None, None, :] + P * np.arange(2)[None, :, None]),
                               (P, 2, P)).astype(bf16).copy()

    nc = bacc.Bacc("TRN2")
    dt = mybir.dt
    t_xlo = nc.declare_dram_parameter("xlo", [XSPLIT, DIN], dt.bfloat16, isOutput=False)
    t_xhi = nc.declare_dram_parameter("xhi", [N - XSPLIT, DIN], dt.bfloat16, isOutput=False)
    t_xoT = nc.declare_dram_parameter("xoT", [P, NPAD], dt.bfloat16, isOutput=False)
    t_i1 = nc.declare_dram_parameter("i1", [P, T16_1], dt.int16, isOutput=False)
    t_d1 = nc.declare_dram_parameter("d1", [P, R1], dt.float32, isOutput=False)
    t_s1 = nc.declare_dram_parameter("s1", [P, R1], dt.float32, isOutput=False)
    t_i2 = nc.declare_dram_parameter("i2", [P, T16_2], dt.int16, isOutput=False)
    t_d2 = nc.declare_dram_parameter("d2", [P, R2], dt.float32, isOutput=False)
    t_w1l = nc.declare_dram_parameter("w1l", [DIN, HID], dt.bfloat16, isOutput=False)
    t_w1r = nc.declare_dram_parameter("w1r", [DIN, HID], dt.bfloat16, isOutput=False)
    t_w2l = nc.declare_dram_parameter("w2l", [P, HID // P, OUT], dt.bfloat16, isOutput=False)
    t_w2r = nc.declare_dram_parameter("w2r", [P, HID // P, OUT], dt.bfloat16, isOutput=False)
    t_b1 = nc.declare_dram_parameter("b1", [P, HID // P], dt.float32, isOutput=False)
    t_b2 = nc.declare_dram_parameter("b2b", [P, OUT], dt.float32, isOutput=False)
    t_rc = nc.declare_dram_parameter("rc", [P, NWIN], dt.float32, isOutput=False)
    t_io = nc.declare_dram_parameter("iopar", [P, 2, P], dt.bfloat16, isOutput=False)
    t_out = nc.declare_dram_parameter("out", [NPAD, OUT], dt.float32, isOutput=True)

    t_p = nc.dram_tensor("ptab", [NPAD, P], dt.bfloat16)
    t_partial = nc.dram_tensor("partial", [NCORES, P, NWIN, OUT], dt.bfloat16)
    t_rsout = nc.dram_tensor("rsout", [P, NWIN, OUT], dt.bfloat16)

    AluOp = mybir.AluOpType
    AF = mybir.ActivationFunctionType

    with tile.TileContext(nc) as tc:
        with tc.tile_pool(name="const", bufs=1) as cpool, \
             tc.tile_pool(name="sm", bufs=6) as spool, \
             tc.tile_pool(name="work", bufs=3) as wpool, \
             tc.tile_pool(name="psumA", bufs=2, space="PSUM") as ppool, \
             tc.tile_pool(name="psumB", bufs=1, space="PSUM") as ppoolb:
            nc.gpsimd.load_library(mlp)
            io_t = cpool.tile([P, 2, P], dt.bfloat16)
            nc.sync.dma_start(io_t[:], t_io[:])
            i1_t = cpool.tile([P, T16_1], dt.int16)
            nc.sync.dma_start(i1_t[:], t_i1[:])
            d1_t = cpool.tile([P, R1], dt.float32)
            nc.sync.dma_start(d1_t[:], t_d1[:])
            s1_t = cpool.tile([P, R1], dt.float32)
            nc.sync.dma_start(s1_t[:], t_s1[:])
            i2_t = cpool.tile([P, T16_2], dt.int16)
            nc.sync.dma_start(i2_t[:], t_i2[:])
            d2_t = cpool.tile([P, R2], dt.float32)
            nc.sync.dma_start(d2_t[:], t_d2[:])
            xoT_t = cpool.tile([P, NPAD], dt.bfloat16)
            nc.sync.dma_start(xoT_t[:], t_xoT[:])
            w1l_t = cpool.tile([DIN, HID], dt.bfloat16)
            nc.sync.dma_start(w1l_t[:], t_w1l[:])
            w1r_t = cpool.tile([DIN, HID], dt.bfloat16)
            nc.sync.dma_start(w1r_t[:], t_w1r[:])
            w2l_t = cpool.tile([P, HID // P, OUT], dt.bfloat16)
            nc.sync.dma_start(w2l_t[:], t_w2l[:])
            w2r_t = cpool.tile([P, HID // P, OUT], dt.bfloat16)
            nc.sync.dma_start(w2r_t[:], t_w2r[:])
            b1_t = cpool.tile([P, HID // P], dt.float32)
            nc.sync.dma_start(b1_t[:], t_b1[:])
            b2_t = cpool.tile([P, OUT], dt.float32)
            nc.sync.dma_start(b2_t[:], t_b2[:])
            rc_t = cpool.tile([P, NWIN], dt.float32)
            nc.sync.dma_start(rc_t[:], t_rc[:])
            qbuf = cpool.tile([P, NWIN, OUT], dt.float32)
            zbuf = cpool.tile([P, NWIN, OUT], dt.float32)
            ebuf = cpool.tile([P, NWIN, OUT], dt.float32)
            ssum = cpool.tile([P, NWIN, 1], dt.float32)
            lsum = cpool.tile([P, NWIN, 1], dt.float32)
            rs_t = cpool.tile([P, NWIN, OUT], dt.bfloat16)

            # persistent double-buffered message tiles; memset once so any
            # never-gathered slot holds finite data (S=0 kills it in matmul)
            NMB = 3
            mbufs = [cpool.tile([P, MR, DIN], dt.bfloat16, name=f"mbuf{i}")
                     for i in range(NMB)]
            for mb in mbufs:
                nc.vector.memset(mb[:], 0.0)
            # p rows accumulate in SBUF; one bulk DRAM write at end of L1
            pbuf = cpool.tile([P, NWIN, P], dt.bfloat16)
            nc.vector.memset(pbuf[:], 0.0)

            def seg_reduce(msg, d_t, par, winfo, out_psum, ncols,
                           sc_t=None, transposed=False):
                """selection-matrix segment sum for one window into out_psum.

                transposed: out[feat, dst] via lhsT=msg; else out[dst, feat].
                sc_t: per-slot scale folded into the selection matrix."""
                tot = sum(nr for (_b, _do, _mo, nr) in winfo)
                done = 0
                for (b, do, mo, nr) in winfo:
                    for r in range(nr):
                        # per-partition fp32 scalars keep the DVE 4x mode
                        S = spool.tile([P, P], dt.bfloat16, tag="S")
                        if sc_t is not None:
                            nc.vector.tensor_scalar(
                                S[:], io_t[:, par, :], d_t[:, do + r:do + r + 1],
                                sc_t[:, do + r:do + r + 1],
                                AluOp.is_equal, AluOp.mult)
                        else:
                            nc.vector.tensor_scalar(
                                S[:], io_t[:, par, :], d_t[:, do + r:do + r + 1],
                                None, AluOp.is_equal)
                        if transposed:
                            nc.tensor.matmul(out_psum[:], lhsT=msg[:, mo + r, :ncols],
                                             rhs=S[:], start=(done == 0),
                                             stop=(done == tot - 1))
                        else:
                            nc.tensor.matmul(out_psum[:], lhsT=S[:],
                                             rhs=msg[:, mo + r, :ncols],
                                             start=(done == 0),
                                             stop=(done == tot - 1))
                        done += 1
                return tot

            # ---------------- Layer 1 ----------------
            for ci, ch in enumerate(l1_chunks):
                msg = mbufs[ci % NMB]
                for (b, i16o, call_len, mo, n_ranks, do) in ch["calls"]:
                    tbl = t_xlo[:] if b == 0 else t_xhi[:]
                    nc.gpsimd.dma_gather(
                        msg[:, mo:mo + n_ranks, :], tbl,
                        i1_t[:, i16o:i16o + call_len // 16],
                        call_len, call_len, DIN, single_packet=False)
                for w in ch["ws"]:
                    winfo = ch["windows"][w]
                    amT = wpool.tile([P, P], dt.bfloat16, tag="amT")
                    if winfo:
                        pagg = ppool.tile([P, P], dt.float32, tag="pagg")
                        seg_reduce(msg, d1_t, w & 1, winfo, pagg, DIN,
                                   sc_t=s1_t, transposed=True)
                        nc.scalar.activation(amT[:], pagg[:], AF.Copy)
                    else:
                        nc.vector.memset(amT[:], 0.0)
                    pq = ppool.tile([P, OUT], dt.float32, tag="pq")
                    qq = ppool.tile([P, OUT], dt.float32, tag="qq")
                    for blk in range(HID // P):
                        ph = ppoolb.tile([P, P], dt.float32, tag="ph", bufs=2)
                        nc.tensor.matmul(ph[:], lhsT=w1l_t[:, blk * P:(blk + 1) * P],
                                         rhs=amT[:], start=True, stop=False)
                        nc.tensor.matmul(ph[:], lhsT=w1r_t[:, blk * P:(blk + 1) * P],
                                         rhs=xoT_t[:, w * P:(w + 1) * P],
                                         start=False, stop=True)
                        hT = wpool.tile([P, P], dt.bfloat16, tag="hT")
                        nc.scalar.activation(hT[:], ph[:], AF.Relu,
                                             bias=b1_t[:, blk:blk + 1])
                        nc.tensor.matmul(pq[:], lhsT=hT[:], rhs=w2l_t[:, blk, :],
                                         start=(blk == 0), stop=(blk == 3))
                        nc.tensor.matmul(qq[:], lhsT=hT[:], rhs=w2r_t[:, blk, :],
                                         start=(blk == 0), stop=(blk == 3))
                    nc.scalar.activation(qbuf[:, w, :], qq[:], AF.Copy)
                    nc.scalar.activation(pbuf[:, w, :OUT], pq[:], AF.Copy)
            nc.sync.dma_start(t_p[:].rearrange("(w p) c -> p w c", p=P), pbuf[:])

            # qbuf += b2 (once)
            nc.vector.tensor_tensor(out=qbuf[:], in0=qbuf[:],
                                    in1=b2_t[:, None, :].to_broadcast([P, NWIN, OUT]),
                                    op=AluOp.add)

            # ---------------- Layer 2 partials ----------------
            for ci, ch in enumerate(l2_chunks):
                msg = mbufs[ci % NMB]
                for (b, i16o, call_len, mo, n_ranks, do) in ch["calls"]:
                    nc.gpsimd.dma_gather(
                        msg[:, mo:mo + n_ranks, :], t_p[:],
                        i2_t[:, i16o:i16o + call_len // 16],
                        call_len, call_len, P, single_packet=False)
                slab, w0l = ch["w0"] // NWIN, ch["w0"] % NWIN
                nwch = len(ch["ws"])
                pst = wpool.tile([P, CW2, OUT], dt.bfloat16, tag="pst")
                for w in ch["ws"]:
                    winfo = ch["windows"][w]
                    pagg = ppool.tile([P, OUT], dt.float32, tag="pagg")
                    if winfo:
                        seg_reduce(msg, d2_t, w & 1, winfo, pagg, OUT)
                        nc.scalar.activation(pst[:, w - ch["w0"], :], pagg[:], AF.Copy)
                    else:
                        nc.vector.memset(pst[:, w - ch["w0"], :], 0.0)
                nc.sync.dma_start(t_partial[slab, :, w0l:w0l + nwch, :],
                                  pst[:, :nwch, :])

            # ---------------- ReduceScatter + output ----------------
            nc.gpsimd.collective_compute(
                "ReduceScatter", AluOp.add, replica_groups=[list(range(NCORES))],
                ins=[t_partial[:]], outs=[t_rsout[:]])
            nc.sync.dma_start(rs_t[:], t_rsout[:])
            nc.vector.tensor_tensor(out=zbuf[:], in0=rs_t[:],
                                    in1=rc_t[:, :, None].to_broadcast([P, NWIN, OUT]),
                                    op=AluOp.mult)
            nc.vector.tensor_tensor(out=zbuf[:], in0=zbuf[:], in1=qbuf[:],
                                    op=AluOp.add)
            nc.scalar.activation(ebuf[:], zbuf[:], AF.Exp)
            nc.vector.tensor_reduce(ssum[:], ebuf[:],
                                    axis=mybir.AxisListType.X, op=AluOp.add)
            nc.scalar.activation(lsum[:], ssum[:], AF.Ln)
            nc.vector.tensor_tensor(out=ebuf[:], in0=zbuf[:],
                                    in1=lsum[:].to_broadcast([P, NWIN, OUT]),
                                    op=AluOp.subtract)
            nc.sync.dma_start(t_out[:].rearrange("(w p) o -> p w o", p=P), ebuf[:])

    nc.compile()

    in_maps = []
    for c in range(NCORES):
        i1a, d1a, s1a = l1_pc[c]
        i2a, d2a, _ = l2_pc[c]
        xoT = np.zeros((P, NPAD), bf16)
        xoT[:, :NLOC] = x[c * NLOC:(c + 1) * NLOC].T.astype(bf16)
        rcf = np.ones(NPAD, np.float32)
        rcf[:NLOC] = recip[c * NLOC:(c + 1) * NLOC]
        rcc = rcf.reshape(NWIN, P).T.copy()
        in_maps.append({
            "xlo": xlo, "xhi": xhi, "xoT": xoT,
            "i1": i1a, "d1": d1a, "s1": s1a, "i2": i2a, "d2": d2a,
            "w1l": np.asarray(w1_l).astype(bf16), "w1r": np.asarray(w1_r).astype(bf16),
            "w2l": np.ascontiguousarray(np.asarray(w2_l).astype(bf16).reshape(HID // P, P, OUT).transpose(1, 0, 2)),
            "w2r": np.ascontiguousarray(np.asarray(w2_r).astype(bf16).reshape(HID // P, P, OUT).transpose(1, 0, 2)),
            "b1": np.asarray(b1, np.float32).reshape(HID // P, P).T.copy(),
            "b2b": b2b_np, "rc": rcc, "iopar": iopar_np,
        })
    res = run_bass_kernel_spmd(nc, in_maps, list(range(NCORES)))
    out = np.concatenate([res.results[c]["out"][:NLOC] for c in range(NCORES)], axis=0)
    kernel.last_results = res
    kernel.last_nc = nc
    return out.astype(np.float32)


# revision 58
# speedup vs baseline: 1.9637x; 1.0481x over previous
"""GraphSAGE (2-layer, mean-agg) Trainium2 Bass kernel, 8-core SPMD.

Layer 1: dst-sharded (6250 nodes/core, 49 windows of 128). Messages x[src]
fetched with gpsimd dma_gather (bf16 tables split at 32768 so indices fit
int16); segment-sum via per-rank selection-matrix matmuls accumulated in PSUM.
Window slots are 16-granular (not 128) inside each chunk call; chunk-relative
int16 drel + per-window iota tables disambiguate shared boundary ranks.

Layer 2: src-sharded. Each core computes p = relu(h) @ w2_l for its own nodes,
writes it to a private DRAM table, gathers its own-src edges' p rows (single
int16 bucket), and accumulates partial dst sums for all 8 slabs of 49 windows.
One bf16 ReduceScatter (4MB -> 0.5MB) replaces the baseline's 4 AllGathers.

Output: z = rs*recip + (h@w2_r + b2); log_softmax computed with batched Exp
over all windows and a single Ln (avoids activation-table reloads); logits
are within +-5 so no max-subtraction is needed.
"""
import numpy as np
import ml_dtypes

N = 50000
E = 800000
DIN, HID, OUT = 128, 512, 40
NCORES = 8
NLOC = N // NCORES          # 6250
P = 128
NWIN = (NLOC + P - 1) // P  # 49
NPAD = NWIN * P             # 6272
XSPLIT = 32768              # x table split for int16 gather indices
CW1 = 5                     # L1 windows per gather chunk
CW2 = 25                    # L2 windows per gather chunk (within a slab)
KB = 4                      # selection-matrix ranks per DVE build op

bf16 = ml_dtypes.bfloat16


def _cdiv(a, b):
    return -(-a // b)


def _wrap_idx(flat):
    """int16 wrapped layout for one gather call: slot i -> [i%16, i//16]."""
    n = len(flat)
    w = flat.astype(np.int16).reshape(n // 16, 16).T.copy()
    return np.tile(w, (8, 1))  # [128, n/16]


def _chunk_ranges(nwin_total, cw, period):
    """Window ranges of <=cw windows that never cross a period boundary.
    The final range is split in two so the post-stream compute tail is
    short."""
    out = []
    for p0 in range(0, nwin_total, period):
        pend = min(p0 + period, nwin_total)
        for w0 in range(p0, pend, cw):
            out.append((w0, min(w0 + cw, pend)))
    a, b = out[-1]
    if b - a > 3:
        mid = (a + b + 1) // 2
        out[-1] = (a, mid)
        out.append((mid, b))
    return out


def _build_layer(edge_core_lists, nwin_total, cw, nbuckets, period=None,
                 edge_scale=None):
    """Common (cross-core) schedule for one layer.

    edge_core_lists: per core, (idx, drel_global, win, bucket) arrays where
    win in [0, nwin_total), drel_global = dst offset within the window space
    (win*128 + in-window row). edge_scale: per core, per-edge scale values
    packed alongside drel (selection-matrix entries become this scale).
    Returns (per-core packed (idx16, d, scale) arrays, chunk descriptors,
    totals).
    """
    counts = np.zeros((NCORES, nwin_total, nbuckets), np.int64)
    for c, (gi, dg, win, bk) in enumerate(edge_core_lists):
        np.add.at(counts[c], (win, bk), 1)
    seg16 = counts.max(axis=0).astype(np.int64)  # [nwin, nb] exact max slots
    # a 128-slot rank must never span 3 windows: with >=128-slot segments a
    # rank touches only adjacent windows, which the parity offset in drel
    # disambiguates (values stay < 256 so they are exact in bf16)
    seg16[seg16 > 0] = np.maximum(seg16[seg16 > 0], P)

    chunks = []
    d_off = 0
    i16_off = 0
    for w0, wend in _chunk_ranges(nwin_total, cw, period or nwin_total):
        ws = list(range(w0, wend))
        calls = []
        windows = {w: [] for w in ws}
        msg_off = 0
        for b in range(nbuckets):
            call_len = _cdiv(int(seg16[ws, b].sum()), 16) * 16
            if call_len == 0:
                continue
            n_ranks = _cdiv(call_len, P)
            off = 0
            for w in ws:
                sl = int(seg16[w, b])
                if sl == 0:
                    continue
                ra, rb = off >> 7, _cdiv(off + sl, P)
                windows[w].append((b, d_off + ra, msg_off + ra, rb - ra))
                off += sl
            calls.append((b, i16_off, call_len, msg_off, n_ranks, d_off))
            d_off += n_ranks
            i16_off += call_len // 16
            msg_off += n_ranks
        chunks.append({
            "w0": w0, "ws": ws, "calls": calls, "windows": windows,
            "n_ranks": msg_off,
        })

    # per-core packed arrays
    per_core = []
    for c, (gi, dg, win, bk) in enumerate(edge_core_lists):
        i16_cols = np.zeros((P, i16_off), np.int16)
        d_cols = np.full((P, d_off), -1, np.float32)
        s_cols = np.zeros((P, d_off), np.float32)
        sv = edge_scale[c] if edge_scale is not None else None
        # bucket edge data sorted by (win, bucket) for slot placement
        order = np.lexsort((bk, win))
        gi_s, dg_s, win_s, bk_s = gi[order], dg[order], win[order], bk[order]
        sv_s = sv[order] if sv is not None else None
        # start index of each (win,bucket) group in sorted arrays
        key = win_s * nbuckets + bk_s
        starts = np.searchsorted(key, np.arange(nwin_total * nbuckets))
        ends = np.searchsorted(key, np.arange(nwin_total * nbuckets), side="right")
        for ch in chunks:
            for (b, i16o, call_len, mo, n_ranks, do) in ch["calls"]:
                flat = np.zeros(n_ranks * P, np.int64)
                drel = np.full(n_ranks * P, -1, np.int64)
                sval = np.zeros(n_ranks * P, np.float32)
                off = 0
                for w in ch["ws"]:
                    sl = int(seg16[w, b])
                    if sl == 0:
                        continue
                    a, e = starts[w * nbuckets + b], ends[w * nbuckets + b]
                    cnt = e - a
                    flat[off:off + cnt] = gi_s[a:e]
                    # window-relative row + parity offset (exact in bf16)
                    drel[off:off + cnt] = (dg_s[a:e] - w * P) + (w & 1) * P
                    if sv_s is not None:
                        sval[off:off + cnt] = sv_s[a:e]
                    off += sl
                i16_cols[:, i16o:i16o + call_len // 16] = _wrap_idx(flat[:call_len])
                d_cols[:, do:do + n_ranks] = drel.reshape(n_ranks, P).T.astype(np.float32)
                s_cols[:, do:do + n_ranks] = sval.reshape(n_ranks, P).T
        per_core.append((i16_cols, d_cols, s_cols))
    return per_core, chunks, i16_off, d_off


def _cdiv_arr(a, b):
    return -(-a // b)


def _balance_perm(src, dst):
    """Per-core permutation of local node rows that balances, for every
    (dst window, src core) pair, the number of incoming edges — shrinking
    the cross-core max() padding in both layers' gather schedules.
    Returns perm[NCORES, NLOC]: original local idx -> permuted row."""
    sc = src // NLOC
    perm = np.zeros((NCORES, NLOC), np.int64)
    for c in range(NCORES):
        lo = c * NLOC
        m = (dst >= lo) & (dst < lo + NLOC)
        # per-node in-degree vector by src core [NLOC, 8]
        vec = np.zeros((NLOC, NCORES), np.int64)
        np.add.at(vec, (dst[m] - lo, sc[m]), 1)
        tot = vec.sum(axis=1)
        order = np.argsort(-tot, kind="stable")
        wsum = np.zeros((NWIN, NCORES), np.int64)
        wcap = np.full(NWIN, P, np.int64)
        wcap[NWIN - 1] = NLOC - (NWIN - 1) * P
        wfill = np.zeros(NWIN, np.int64)
        for node in order:
            cand = np.flatnonzero(wfill < wcap[:len(wfill)])
            costs = (wsum[cand] + vec[node]).max(axis=1)
            w = cand[np.argmin(costs)]
            wsum[w] += vec[node]
            perm[c, node] = w * P + wfill[w]
            wfill[w] += 1
    return perm


def _build_schedule(edge_index):
    src = np.asarray(edge_index[0], dtype=np.int64)
    dst = np.asarray(edge_index[1], dtype=np.int64)
    deg = np.bincount(dst, minlength=N).astype(np.float32)
    recip = 1.0 / np.maximum(deg, 1.0)
    perm = _balance_perm(src, dst)

    # ---- L1: dst-sharded; selection entries carry 1/deg so the PSUM sum
    # is already the mean ----
    l1_lists, l1_scales = [], []
    for c in range(NCORES):
        lo, hi = c * NLOC, (c + 1) * NLOC
        m = (dst >= lo) & (dst < hi)
        s, dg = src[m], perm[c][dst[m] - lo]
        bk = (s >= XSPLIT).astype(np.int64)
        gi = np.where(bk == 1, s - XSPLIT, s)
        l1_lists.append((gi, dg, dg >> 7, bk))
        l1_scales.append(recip[dst[m]])
    l1_pc, l1_chunks, T16_1, R1 = _build_layer(l1_lists, NWIN, CW1, 2,
                                               edge_scale=l1_scales)

    # ---- L2: src-sharded, windows = slab*NWIN + within-slab window ----
    l2_lists = []
    for c in range(NCORES):
        lo, hi = c * NLOC, (c + 1) * NLOC
        m = (src >= lo) & (src < hi)
        s, d = perm[c][src[m] - lo], dst[m]
        slab = d // NLOC
        pr = perm[slab, d - slab * NLOC]
        win = slab * NWIN + (pr >> 7)
        dg = win * P + (pr & (P - 1))
        l2_lists.append((s, dg, win, np.zeros(len(s), np.int64)))
    l2_pc, l2_chunks, T16_2, R2 = _build_layer(l2_lists, NWIN * NCORES, CW2, 1,
                                               period=NWIN)

    return (l1_pc, l1_chunks, T16_1, R1, l2_pc, l2_chunks, T16_2, R2, recip,
            perm)


def kernel(x, edge_index, w1_l, b1, w1_r, w2_l, b2, w2_r):
    import concourse.bacc as bacc
    import concourse.mybir as mybir
    import concourse.tile as tile
    from concourse.bass_utils import run_bass_kernel_spmd
    from concourse.library_config import mlp
    from concourse.masks import make_identity

    x = np.asarray(x, np.float32)
    (l1_pc, l1_chunks, T16_1, R1, l2_pc, l2_chunks, T16_2, R2, recip,
     perm) = _build_schedule(np.asarray(edge_index))

    MR1 = max(ch["n_ranks"] for ch in l1_chunks)   # msg tile ranks (L1)
    MR2 = max(ch["n_ranks"] for ch in l2_chunks)
    MR = max(MR1, MR2)

    xlo = np.ascontiguousarray(x[:XSPLIT].astype(bf16))
    xhi = np.ascontiguousarray(x[XSPLIT:].astype(bf16))
    b2b_np = np.tile(np.asarray(b2, np.float32)[None, :], (P, 1))
    iota = np.arange(P, dtype=np.int64)
    iopar_np = np.broadcast_to((iota[None, None, :] + P * np.arange(2)[None, :, None]),
                               (P, 2, P)).astype(bf16).copy()

    nc = bacc.Bacc("TRN2")
    dt = mybir.dt
    t_xlo = nc.declare_dram_parameter("xlo", [XSPLIT, DIN], dt.bfloat16, isOutput=False)
    t_xhi = nc.declare_dram_parameter("xhi", [N - XSPLIT, DIN], dt.bfloat16, isOutput=False)
    t_xoT = nc.declare_dram_parameter("xoT", [P, NPAD], dt.bfloat16, isOutput=False)
    t_i1 = nc.declare_dram_parameter("i1", [P, T16_1], dt.int16, isOutput=False)
    t_d1 = nc.declare_dram_parameter("d1", [P, R1], dt.float32, isOutput=False)
    t_s1 = nc.declare_dram_parameter("s1", [P, R1], dt.float32, isOutput=False)
    t_i2 = nc.declare_dram_parameter("i2", [P, T16_2], dt.int16, isOutput=False)
    t_d2 = nc.declare_dram_parameter("d2", [P, R2], dt.float32, isOutput=False)
    t_w1l = nc.declare_dram_parameter("w1l", [DIN, HID], dt.bfloat16, isOutput=False)
    t_w1r = nc.declare_dram_parameter("w1r", [DIN, HID], dt.bfloat16, isOutput=False)
    t_w2l = nc.declare_dram_parameter("w2l", [P, HID // P, OUT], dt.bfloat16, isOutput=False)
    t_w2r = nc.declare_dram_parameter("w2r", [P, HID // P, OUT], dt.bfloat16, isOutput=False)
    t_b1 = nc.declare_dram_parameter("b1", [P, HID // P], dt.float32, isOutput=False)
    t_b2 = nc.declare_dram_parameter("b2b", [P, OUT], dt.float32, isOutput=False)
    t_rc = nc.declare_dram_parameter("rc", [P, NWIN], dt.float32, isOutput=False)
    t_io = nc.declare_dram_parameter("iopar", [P, 2, P], dt.bfloat16, isOutput=False)
    t_out = nc.declare_dram_parameter("out", [NPAD, OUT], dt.float32, isOutput=True)

    t_p = nc.dram_tensor("ptab", [NPAD, P], dt.bfloat16)
    t_partial = nc.dram_tensor("partial", [NCORES, P, NWIN, OUT], dt.bfloat16)
    t_rsout = nc.dram_tensor("rsout", [P, NWIN, OUT], dt.bfloat16)

    AluOp = mybir.AluOpType
    AF = mybir.ActivationFunctionType

    with tile.TileContext(nc) as tc:
        with tc.tile_pool(name="const", bufs=1) as cpool, \
             tc.tile_pool(name="sm", bufs=6) as spool, \
             tc.tile_pool(name="work", bufs=3) as wpool, \
             tc.tile_pool(name="psumA", bufs=2, space="PSUM") as ppool, \
             tc.tile_pool(name="psumB", bufs=1, space="PSUM") as ppoolb:
            nc.gpsimd.load_library(mlp)
            # L1-critical loads first so the first gather can launch early
            io_t = cpool.tile([P, 2, P], dt.bfloat16)
            nc.sync.dma_start(io_t[:], t_io[:])
            i1_t = cpool.tile([P, T16_1], dt.int16)
            nc.sync.dma_start(i1_t[:], t_i1[:])
            d1_t = cpool.tile([P, R1], dt.float32)
            nc.sync.dma_start(d1_t[:], t_d1[:])
            s1_t = cpool.tile([P, R1], dt.float32)
            nc.sync.dma_start(s1_t[:], t_s1[:])
            # message tiles; memset once so any never-gathered slot holds
            # finite data (S=0 kills it in the matmul); buf 0 first
            NMB = 3
            mbufs = [cpool.tile([P, MR, DIN], dt.bfloat16, name=f"mbuf{i}")
                     for i in range(NMB)]
            mh = MR // 2
            nc.vector.memset(mbufs[0][:, :mh, :], 0.0)
            nc.gpsimd.memset(mbufs[0][:, mh:, :], 0.0)
            nc.vector.memset(mbufs[1][:], 0.0)
            nc.gpsimd.memset(mbufs[2][:], 0.0)
            xoT_t = cpool.tile([P, NPAD], dt.bfloat16)
            nc.sync.dma_start(xoT_t[:], t_xoT[:])
            w1l_t = cpool.tile([DIN, HID], dt.bfloat16)
            nc.sync.dma_start(w1l_t[:], t_w1l[:])
            w1r_t = cpool.tile([DIN, HID], dt.bfloat16)
            nc.sync.dma_start(w1r_t[:], t_w1r[:])
            w2l_t = cpool.tile([P, HID // P, OUT], dt.bfloat16)
            nc.sync.dma_start(w2l_t[:], t_w2l[:])
            w2r_t = cpool.tile([P, HID // P, OUT], dt.bfloat16)
            nc.sync.dma_start(w2r_t[:], t_w2r[:])
            b1_t = cpool.tile([P, HID // P], dt.float32)
            nc.sync.dma_start(b1_t[:], t_b1[:])
            b2_t = cpool.tile([P, OUT], dt.float32)
            nc.sync.dma_start(b2_t[:], t_b2[:])
            rc_t = cpool.tile([P, NWIN], dt.float32)
            nc.sync.dma_start(rc_t[:], t_rc[:])
            qbuf = cpool.tile([P, NWIN, OUT], dt.float32)
            zbuf = cpool.tile([P, NWIN, OUT], dt.float32)
            ebuf = cpool.tile([P, NWIN, OUT], dt.float32)
            ssum = cpool.tile([P, NWIN, 1], dt.float32)
            lsum = cpool.tile([P, NWIN, 1], dt.float32)
            rs_t = cpool.tile([P, NWIN, OUT], dt.bfloat16)
            # p rows accumulate in SBUF; one bulk DRAM write at end of L1
            pbuf = cpool.tile([P, NWIN, P], dt.bfloat16)
            nc.vector.memset(pbuf[:], 0.0)

            def seg_reduce(msg, d_t, par, winfo, out_psum, ncols,
                           sc_t=None, transposed=False):
                """selection-matrix segment sum for one window into out_psum.

                transposed: out[feat, dst] via lhsT=msg; else out[dst, feat].
                sc_t: per-slot scale folded into the selection matrix."""
                tot = sum(nr for (_b, _do, _mo, nr) in winfo)
                done = 0
                for (b, do, mo, nr) in winfo:
                    for r in range(nr):
                        # per-partition fp32 scalars keep the DVE 4x mode
                        S = spool.tile([P, P], dt.bfloat16, tag="S")
                        if sc_t is not None:
                            nc.vector.tensor_scalar(
                                S[:], io_t[:, par, :], d_t[:, do + r:do + r + 1],
                                sc_t[:, do + r:do + r + 1],
                                AluOp.is_equal, AluOp.mult)
                        else:
                            nc.vector.tensor_scalar(
                                S[:], io_t[:, par, :], d_t[:, do + r:do + r + 1],
                                None, AluOp.is_equal)
                        if transposed:
                            nc.tensor.matmul(out_psum[:], lhsT=msg[:, mo + r, :ncols],
                                             rhs=S[:], start=(done == 0),
                                             stop=(done == tot - 1))
                        else:
                            nc.tensor.matmul(out_psum[:], lhsT=S[:],
                                             rhs=msg[:, mo + r, :ncols],
                                             start=(done == 0),
                                             stop=(done == tot - 1))
                        done += 1
                return tot

            # ---------------- Layer 1 ----------------
            i2_t = cpool.tile([P, T16_2], dt.int16)
            d2_t = cpool.tile([P, R2], dt.float32)
            for ci, ch in enumerate(l1_chunks):
                msg = mbufs[ci % NMB]
                for (b, i16o, call_len, mo, n_ranks, do) in ch["calls"]:
                    tbl = t_xlo[:] if b == 0 else t_xhi[:]
                    nc.gpsimd.dma_gather(
                        msg[:, mo:mo + n_ranks, :], tbl,
                        i1_t[:, i16o:i16o + call_len // 16],
                        call_len, call_len, DIN, single_packet=False)
                if ci == 1:
                    # L2 index loads deferred past the first gathers
                    nc.sync.dma_start(i2_t[:], t_i2[:])
                    nc.sync.dma_start(d2_t[:], t_d2[:])
                # phase 1: aggregation + h blocks (sels -> amT -> ph -> hT);
                # phase 2: output projections. Splitting keeps the PE queue
                # free of long Act-dependent stalls between windows.
                hts = {}
                for w in ch["ws"]:
                    winfo = ch["windows"][w]
                    amT = wpool.tile([P, P], dt.bfloat16, tag="amT")
                    if winfo:
                        pagg = ppool.tile([P, P], dt.float32, tag="pagg")
                        seg_reduce(msg, d1_t, w & 1, winfo, pagg, DIN,
                                   sc_t=s1_t, transposed=True)
                        nc.scalar.activation(amT[:], pagg[:], AF.Copy)
                    else:
                        nc.vector.memset(amT[:], 0.0)
                    hts[w] = []
                    for blk in range(HID // P):
                        ph = ppoolb.tile([P, P], dt.float32, tag="ph", bufs=2)
                        nc.tensor.matmul(ph[:], lhsT=w1l_t[:, blk * P:(blk + 1) * P],
                                         rhs=amT[:], start=True, stop=False)
                        nc.tensor.matmul(ph[:], lhsT=w1r_t[:, blk * P:(blk + 1) * P],
                                         rhs=xoT_t[:, w * P:(w + 1) * P],
                                         start=False, stop=True)
                        hT = wpool.tile([P, P], dt.bfloat16, tag="hT",
                                        bufs=4 * CW1 + 2)
                        nc.scalar.activation(hT[:], ph[:], AF.Relu,
                                             bias=b1_t[:, blk:blk + 1])
                        hts[w].append(hT)
                for w in ch["ws"]:
                    pq = ppool.tile([P, OUT], dt.float32, tag="pq")
                    qq = ppool.tile([P, OUT], dt.float32, tag="qq")
                    for blk in range(HID // P):
                        hT = hts[w][blk]
                        nc.tensor.matmul(pq[:], lhsT=hT[:], rhs=w2l_t[:, blk, :],
                                         start=(blk == 0), stop=(blk == 3))
                        nc.tensor.matmul(qq[:], lhsT=hT[:], rhs=w2r_t[:, blk, :],
                                         start=(blk == 0), stop=(blk == 3))
                    nc.scalar.activation(qbuf[:, w, :], qq[:], AF.Copy)
                    nc.scalar.activation(pbuf[:, w, :OUT], pq[:], AF.Copy)
            nc.sync.dma_start(t_p[:].rearrange("(w p) c -> p w c", p=P), pbuf[:])

            # qbuf += b2 (once)
            nc.vector.tensor_tensor(out=qbuf[:], in0=qbuf[:],
                                    in1=b2_t[:, None, :].to_broadcast([P, NWIN, OUT]),
                                    op=AluOp.add)

            # ---------------- Layer 2 partials ----------------
            for ci, ch in enumerate(l2_chunks):
                msg = mbufs[ci % NMB]
                for (b, i16o, call_len, mo, n_ranks, do) in ch["calls"]:
                    nc.gpsimd.dma_gather(
                        msg[:, mo:mo + n_ranks, :], t_p[:],
                        i2_t[:, i16o:i16o + call_len // 16],
                        call_len, call_len, P, single_packet=False)
                slab, w0l = ch["w0"] // NWIN, ch["w0"] % NWIN
                nwch = len(ch["ws"])
                pst = wpool.tile([P, CW2, OUT], dt.bfloat16, tag="pst")
                for w in ch["ws"]:
                    winfo = ch["windows"][w]
                    pagg = ppool.tile([P, OUT], dt.float32, tag="pagg")
                    if winfo:
                        seg_reduce(msg, d2_t, w & 1, winfo, pagg, OUT)
                        nc.scalar.activation(pst[:, w - ch["w0"], :], pagg[:], AF.Copy)
                    else:
                        nc.vector.memset(pst[:, w - ch["w0"], :], 0.0)
                nc.sync.dma_start(t_partial[slab, :, w0l:w0l + nwch, :],
                                  pst[:, :nwch, :])

            # ---------------- ReduceScatter + output ----------------
            nc.gpsimd.collective_compute(
                "ReduceScatter", AluOp.add, replica_groups=[list(range(NCORES))],
                ins=[t_partial[:]], outs=[t_rsout[:]])
            nc.sync.dma_start(rs_t[:], t_rsout[:])
            # pipelined in window halves to shorten the serial chain
            halves = [(0, NWIN // 2), (NWIN // 2, NWIN)]
            for a, b in halves:
                nc.vector.tensor_tensor(
                    out=zbuf[:, a:b, :], in0=rs_t[:, a:b, :],
                    in1=rc_t[:, a:b, None].to_broadcast([P, b - a, OUT]),
                    op=AluOp.mult)
                nc.vector.tensor_tensor(out=zbuf[:, a:b, :], in0=zbuf[:, a:b, :],
                                        in1=qbuf[:, a:b, :], op=AluOp.add)
                nc.scalar.activation(ebuf[:, a:b, :], zbuf[:, a:b, :], AF.Exp)
                nc.vector.tensor_reduce(ssum[:, a:b, :], ebuf[:, a:b, :],
                                        axis=mybir.AxisListType.X, op=AluOp.add)
            nc.scalar.activation(lsum[:], ssum[:], AF.Ln)
            for a, b in halves:
                nc.vector.tensor_tensor(
                    out=ebuf[:, a:b, :], in0=zbuf[:, a:b, :],
                    in1=lsum[:, a:b, :].to_broadcast([P, b - a, OUT]),
                    op=AluOp.subtract)
            nc.sync.dma_start(t_out[:].rearrange("(w p) o -> p w o", p=P), ebuf[:])

    nc.compile()

    in_maps = []
    for c in range(NCORES):
        i1a, d1a, s1a = l1_pc[c]
        i2a, d2a, _ = l2_pc[c]
        xoT = np.zeros((P, NPAD), bf16)
        xoT[:, perm[c]] = x[c * NLOC:(c + 1) * NLOC].T.astype(bf16)
        rcf = np.ones(NPAD, np.float32)
        rcf[perm[c]] = recip[c * NLOC:(c + 1) * NLOC]
        rcc = rcf.reshape(NWIN, P).T.copy()
        in_maps.append({
            "xlo": xlo, "xhi": xhi, "xoT": xoT,
            "i1": i1a, "d1": d1a, "s1": s1a, "i2": i2a, "d2": d2a,
            "w1l": np.asarray(w1_l).astype(bf16), "w1r": np.asarray(w1_r).astype(bf16),
            "w2l": np.ascontiguousarray(np.asarray(w2_l).astype(bf16).reshape(HID // P, P, OUT).transpose(1, 0, 2)),
            "w2r": np.ascontiguousarray(np.asarray(w2_r).astype(bf16).reshape(HID // P, P, OUT).transpose(1, 0, 2)),
            "b1": np.asarray(b1, np.float32).reshape(HID // P, P).T.copy(),
            "b2b": b2b_np, "rc": rcc, "iopar": iopar_np,
        })
    res = run_bass_kernel_spmd(nc, in_maps, list(range(NCORES)))
    out = np.concatenate([res.results[c]["out"][perm[c]] for c in range(NCORES)],
                         axis=0)
    kernel.last_results = res
    kernel.last_nc = nc
    return out.astype(np.float32)


# revision 77
# speedup vs baseline: 2.0006x; 1.0188x over previous
"""GraphSAGE (2-layer, mean-agg) Trainium2 Bass kernel, 8-core SPMD.

Layer 1: dst-sharded (6250 nodes/core, 49 windows of 128). Messages x[src]
fetched with gpsimd dma_gather (bf16 tables split at 32768 so indices fit
int16); segment-sum via per-rank selection-matrix matmuls accumulated in PSUM.
Window slots are 16-granular (not 128) inside each chunk call; chunk-relative
int16 drel + per-window iota tables disambiguate shared boundary ranks.

Layer 2: src-sharded. Each core computes p = relu(h) @ w2_l for its own nodes,
writes it to a private DRAM table, gathers its own-src edges' p rows (single
int16 bucket), and accumulates partial dst sums for all 8 slabs of 49 windows.
One bf16 ReduceScatter (4MB -> 0.5MB) replaces the baseline's 4 AllGathers.

Output: z = rs*recip + (h@w2_r + b2); log_softmax computed with batched Exp
over all windows and a single Ln (avoids activation-table reloads); logits
are within +-5 so no max-subtraction is needed.
"""
import numpy as np
import ml_dtypes

N = 50000
E = 800000
DIN, HID, OUT = 128, 512, 40
NCORES = 8
NLOC = N // NCORES          # 6250
P = 128
NWIN = (NLOC + P - 1) // P  # 49
NPAD = NWIN * P             # 6272
XSPLIT = 32768              # x table split for int16 gather indices
CW1 = 5                     # L1 windows per gather chunk
CW2 = 25                    # L2 windows per gather chunk (within a slab)
KB = 4                      # selection-matrix ranks per DVE build op

bf16 = ml_dtypes.bfloat16


def _cdiv(a, b):
    return -(-a // b)


def _wrap_idx(flat):
    """int16 wrapped layout for one gather call: slot i -> [i%16, i//16]."""
    n = len(flat)
    w = flat.astype(np.int16).reshape(n // 16, 16).T.copy()
    return np.tile(w, (8, 1))  # [128, n/16]


def _chunk_ranges(nwin_total, cw, period, tail):
    """Window ranges of <=cw windows that never cross a period boundary.
    The final range is re-split into ranges of <=tail windows so the
    post-stream compute drain is short."""
    out = []
    for p0 in range(0, nwin_total, period):
        pend = min(p0 + period, nwin_total)
        for w0 in range(p0, pend, cw):
            out.append((w0, min(w0 + cw, pend)))
    a, b = out.pop()
    for w0 in range(a, b, tail):
        out.append((w0, min(w0 + tail, b)))
    return out


def _build_layer(edge_core_lists, nwin_total, cw, nbuckets, period=None,
                 edge_scale=None, tail=None, full_pad_chunks=0):
    """Common (cross-core) schedule for one layer.

    edge_core_lists: per core, (idx, drel_global, win, bucket) arrays where
    win in [0, nwin_total), drel_global = dst offset within the window space
    (win*128 + in-window row). edge_scale: per core, per-edge scale values
    packed alongside drel (selection-matrix entries become this scale).
    Returns (per-core packed (idx16, d, scale) arrays, chunk descriptors,
    totals).
    """
    counts = np.zeros((NCORES, nwin_total, nbuckets), np.int64)
    for c, (gi, dg, win, bk) in enumerate(edge_core_lists):
        np.add.at(counts[c], (win, bk), 1)
    seg16 = counts.max(axis=0).astype(np.int64)  # [nwin, nb] exact max slots
    # a 128-slot rank must never span 3 windows: with >=128-slot segments a
    # rank touches only adjacent windows, which the parity offset in drel
    # disambiguates (values stay < 256 so they are exact in bf16)
    seg16[seg16 > 0] = np.maximum(seg16[seg16 > 0], P)

    chunks = []
    d_off = 0
    i16_off = 0
    for ci, (w0, wend) in enumerate(_chunk_ranges(nwin_total, cw,
                                                  period or nwin_total,
                                                  tail or max(2, cw // 2))):
        ws = list(range(w0, wend))
        calls = []
        windows = {w: [] for w in ws}
        msg_off = 0
        for b in range(nbuckets):
            # first-use chunks of each message buffer gather full ranks so
            # no SBUF slot is ever read uninitialized (pad idx 0, drel -1)
            gran = P if ci < full_pad_chunks else 16
            call_len = _cdiv(int(seg16[ws, b].sum()), gran) * gran
            if call_len == 0:
                continue
            n_ranks = _cdiv(call_len, P)
            off = 0
            for w in ws:
                sl = int(seg16[w, b])
                if sl == 0:
                    continue
                ra, rb = off >> 7, _cdiv(off + sl, P)
                windows[w].append((b, d_off + ra, msg_off + ra, rb - ra))
                off += sl
            calls.append((b, i16_off, call_len, msg_off, n_ranks, d_off))
            d_off += n_ranks
            i16_off += call_len // 16
            msg_off += n_ranks
        chunks.append({
            "w0": w0, "ws": ws, "calls": calls, "windows": windows,
            "n_ranks": msg_off,
        })

    # per-core packed arrays
    per_core = []
    for c, (gi, dg, win, bk) in enumerate(edge_core_lists):
        i16_cols = np.zeros((P, i16_off), np.int16)
        d_cols = np.full((P, d_off), -1, np.float32)
        s_cols = np.zeros((P, d_off), np.float32)
        sv = edge_scale[c] if edge_scale is not None else None
        # bucket edge data sorted by (win, bucket) for slot placement
        order = np.lexsort((bk, win))
        gi_s, dg_s, win_s, bk_s = gi[order], dg[order], win[order], bk[order]
        sv_s = sv[order] if sv is not None else None
        # start index of each (win,bucket) group in sorted arrays
        key = win_s * nbuckets + bk_s
        starts = np.searchsorted(key, np.arange(nwin_total * nbuckets))
        ends = np.searchsorted(key, np.arange(nwin_total * nbuckets), side="right")
        for ch in chunks:
            for (b, i16o, call_len, mo, n_ranks, do) in ch["calls"]:
                flat = np.zeros(n_ranks * P, np.int64)
                drel = np.full(n_ranks * P, -1, np.int64)
                sval = np.zeros(n_ranks * P, np.float32)
                off = 0
                for w in ch["ws"]:
                    sl = int(seg16[w, b])
                    if sl == 0:
                        continue
                    a, e = starts[w * nbuckets + b], ends[w * nbuckets + b]
                    cnt = e - a
                    flat[off:off + cnt] = gi_s[a:e]
                    # window-relative row + parity offset (exact in bf16)
                    drel[off:off + cnt] = (dg_s[a:e] - w * P) + (w & 1) * P
                    if sv_s is not None:
                        sval[off:off + cnt] = sv_s[a:e]
                    off += sl
                i16_cols[:, i16o:i16o + call_len // 16] = _wrap_idx(flat[:call_len])
                d_cols[:, do:do + n_ranks] = drel.reshape(n_ranks, P).T.astype(np.float32)
                s_cols[:, do:do + n_ranks] = sval.reshape(n_ranks, P).T
        per_core.append((i16_cols, d_cols, s_cols))
    return per_core, chunks, i16_off, d_off


def _cdiv_arr(a, b):
    return -(-a // b)


def _balance_perm(src, dst):
    """Per-core permutation of local node rows that balances, for every
    (dst window, src core) pair, the number of incoming edges — shrinking
    the cross-core max() padding in both layers' gather schedules.
    Returns perm[NCORES, NLOC]: original local idx -> permuted row."""
    sc = src // NLOC
    perm = np.zeros((NCORES, NLOC), np.int64)
    for c in range(NCORES):
        lo = c * NLOC
        m = (dst >= lo) & (dst < lo + NLOC)
        # per-node in-degree vector by src core [NLOC, 8]
        vec = np.zeros((NLOC, NCORES), np.int64)
        np.add.at(vec, (dst[m] - lo, sc[m]), 1)
        tot = vec.sum(axis=1)
        order = np.argsort(-tot, kind="stable")
        wsum = np.zeros((NWIN, NCORES), np.int64)
        wcap = np.full(NWIN, P, np.int64)
        wcap[NWIN - 1] = NLOC - (NWIN - 1) * P
        wfill = np.zeros(NWIN, np.int64)
        for node in order:
            cand = np.flatnonzero(wfill < wcap[:len(wfill)])
            costs = (wsum[cand] + vec[node]).max(axis=1)
            w = cand[np.argmin(costs)]
            wsum[w] += vec[node]
            perm[c, node] = w * P + wfill[w]
            wfill[w] += 1
    return perm


def _build_schedule(edge_index):
    src = np.asarray(edge_index[0], dtype=np.int64)
    dst = np.asarray(edge_index[1], dtype=np.int64)
    deg = np.bincount(dst, minlength=N).astype(np.float32)
    recip = 1.0 / np.maximum(deg, 1.0)
    perm = _balance_perm(src, dst)

    # ---- L1: dst-sharded; selection entries carry 1/deg so the PSUM sum
    # is already the mean ----
    l1_lists, l1_scales = [], []
    for c in range(NCORES):
        lo, hi = c * NLOC, (c + 1) * NLOC
        m = (dst >= lo) & (dst < hi)
        s, dg = src[m], perm[c][dst[m] - lo]
        bk = (s >= XSPLIT).astype(np.int64)
        gi = np.where(bk == 1, s - XSPLIT, s)
        l1_lists.append((gi, dg, dg >> 7, bk))
        l1_scales.append(recip[dst[m]])
    l1_pc, l1_chunks, T16_1, R1 = _build_layer(l1_lists, NWIN, CW1, 2,
                                               edge_scale=l1_scales, tail=4,
                                               full_pad_chunks=4)

    # ---- L2: src-sharded, windows = slab*NWIN + within-slab window ----
    l2_lists = []
    for c in range(NCORES):
        lo, hi = c * NLOC, (c + 1) * NLOC
        m = (src >= lo) & (src < hi)
        s, d = perm[c][src[m] - lo], dst[m]
        slab = d // NLOC
        pr = perm[slab, d - slab * NLOC]
        win = slab * NWIN + (pr >> 7)
        dg = win * P + (pr & (P - 1))
        l2_lists.append((s, dg, win, np.zeros(len(s), np.int64)))
    l2_pc, l2_chunks, T16_2, R2 = _build_layer(l2_lists, NWIN * NCORES, CW2, 1,
                                               period=NWIN, tail=3)

    return (l1_pc, l1_chunks, T16_1, R1, l2_pc, l2_chunks, T16_2, R2, recip,
            perm)


def kernel(x, edge_index, w1_l, b1, w1_r, w2_l, b2, w2_r):
    import concourse.bacc as bacc
    import concourse.mybir as mybir
    import concourse.tile as tile
    from concourse.bass_utils import run_bass_kernel_spmd
    from concourse.library_config import mlp
    from concourse.masks import make_identity

    x = np.asarray(x, np.float32)
    (l1_pc, l1_chunks, T16_1, R1, l2_pc, l2_chunks, T16_2, R2, recip,
     perm) = _build_schedule(np.asarray(edge_index))

    MR1 = max(ch["n_ranks"] for ch in l1_chunks)   # msg tile ranks (L1)
    MR2 = max(ch["n_ranks"] for ch in l2_chunks)
    MR = max(MR1, MR2)

    xlo = np.ascontiguousarray(x[:XSPLIT].astype(bf16))
    xhi = np.ascontiguousarray(x[XSPLIT:].astype(bf16))
    b2b_np = np.tile(np.asarray(b2, np.float32)[None, :], (P, 1))
    iota = np.arange(P, dtype=np.int64)
    iopar_np = np.broadcast_to((iota[None, None, :] + P * np.arange(2)[None, :, None]),
                               (P, 2, P)).astype(bf16).copy()

    nc = bacc.Bacc("TRN2")
    dt = mybir.dt
    t_xlo = nc.declare_dram_parameter("xlo", [XSPLIT, DIN], dt.bfloat16, isOutput=False)
    t_xhi = nc.declare_dram_parameter("xhi", [N - XSPLIT, DIN], dt.bfloat16, isOutput=False)
    t_xoT = nc.declare_dram_parameter("xoT", [P, NPAD], dt.bfloat16, isOutput=False)
    t_i1 = nc.declare_dram_parameter("i1", [P, T16_1], dt.int16, isOutput=False)
    t_d1 = nc.declare_dram_parameter("d1", [P, R1], dt.float32, isOutput=False)
    t_s1 = nc.declare_dram_parameter("s1", [P, R1], dt.float32, isOutput=False)
    t_i2 = nc.declare_dram_parameter("i2", [P, T16_2], dt.int16, isOutput=False)
    t_d2 = nc.declare_dram_parameter("d2", [P, R2], dt.float32, isOutput=False)
    t_w1l = nc.declare_dram_parameter("w1l", [DIN, HID], dt.bfloat16, isOutput=False)
    t_w1r = nc.declare_dram_parameter("w1r", [DIN, HID], dt.bfloat16, isOutput=False)
    t_w2l = nc.declare_dram_parameter("w2l", [P, HID // P, OUT], dt.bfloat16, isOutput=False)
    t_w2r = nc.declare_dram_parameter("w2r", [P, HID // P, OUT], dt.bfloat16, isOutput=False)
    t_b1 = nc.declare_dram_parameter("b1", [P, HID // P], dt.float32, isOutput=False)
    t_b2 = nc.declare_dram_parameter("b2b", [P, OUT], dt.float32, isOutput=False)
    t_rc = nc.declare_dram_parameter("rc", [P, NWIN], dt.float32, isOutput=False)
    t_io = nc.declare_dram_parameter("iopar", [P, 2, P], dt.bfloat16, isOutput=False)
    t_out = nc.declare_dram_parameter("out", [NPAD, OUT], dt.float32, isOutput=True)

    t_p = nc.dram_tensor("ptab", [NPAD, P], dt.bfloat16)
    t_partial = nc.dram_tensor("partial", [NCORES, P, NWIN, OUT], dt.bfloat16)
    t_rsout = nc.dram_tensor("rsout", [P, NWIN, OUT], dt.bfloat16)

    AluOp = mybir.AluOpType
    AF = mybir.ActivationFunctionType

    with tile.TileContext(nc) as tc:
        with tc.tile_pool(name="const", bufs=1) as cpool, \
             tc.tile_pool(name="sm", bufs=6) as spool, \
             tc.tile_pool(name="work", bufs=3) as wpool, \
             tc.tile_pool(name="psumA", bufs=2, space="PSUM") as ppool, \
             tc.tile_pool(name="psumB", bufs=1, space="PSUM") as ppoolb:
            nc.gpsimd.load_library(mlp)
            # i1 first: it gates the first gather's descriptor generation
            i1_t = cpool.tile([P, T16_1], dt.int16)
            nc.sync.dma_start(i1_t[:], t_i1[:])
            io_t = cpool.tile([P, 2, P], dt.bfloat16)
            nc.sync.dma_start(io_t[:], t_io[:])
            d1_t = cpool.tile([P, R1], dt.float32)
            nc.sync.dma_start(d1_t[:], t_d1[:])
            s1_t = cpool.tile([P, R1], dt.float32)
            nc.sync.dma_start(s1_t[:], t_s1[:])
            xoT_t = cpool.tile([P, NPAD], dt.bfloat16)
            nc.sync.dma_start(xoT_t[:], t_xoT[:])
            b1_t = cpool.tile([P, HID // P], dt.float32)
            nc.sync.dma_start(b1_t[:], t_b1[:])
            b2_t = cpool.tile([P, OUT], dt.float32)
            nc.sync.dma_start(b2_t[:], t_b2[:])
            rc_t = cpool.tile([P, NWIN], dt.float32)
            nc.sync.dma_start(rc_t[:], t_rc[:])
            # message tiles: the first-use gather of buf i covers ranks
            # [0, len_i); only the residue needs zeroing (finite data so
            # S=0 kills any junk in the matmul)
            NMB = 4
            mbufs = [cpool.tile([P, MR, DIN], dt.bfloat16, name=f"mbuf{i}")
                     for i in range(NMB)]
            for i in range(NMB):
                ln = l1_chunks[i]["n_ranks"] if i < len(l1_chunks) else 0
                if ln < MR:
                    nc.vector.memset(mbufs[i][:, ln:, :], 0.0)
            w1l_t = cpool.tile([DIN, HID], dt.bfloat16)
            nc.sync.dma_start(w1l_t[:], t_w1l[:])
            w1r_t = cpool.tile([DIN, HID], dt.bfloat16)
            nc.sync.dma_start(w1r_t[:], t_w1r[:])
            w2l_t = cpool.tile([P, HID // P, OUT], dt.bfloat16)
            nc.sync.dma_start(w2l_t[:], t_w2l[:])
            w2r_t = cpool.tile([P, HID // P, OUT], dt.bfloat16)
            nc.sync.dma_start(w2r_t[:], t_w2r[:])
            qbuf = cpool.tile([P, NWIN, OUT], dt.float32)
            zbuf = cpool.tile([P, NWIN, OUT], dt.float32)
            ebuf = cpool.tile([P, NWIN, OUT], dt.float32)
            ssum = cpool.tile([P, NWIN, 1], dt.float32)
            lsum = cpool.tile([P, NWIN, 1], dt.float32)
            rs_t = cpool.tile([P, NWIN, OUT], dt.bfloat16)
            # p rows accumulate in SBUF; one bulk DRAM write at end of L1
            pbuf = cpool.tile([P, NWIN, OUT], dt.bfloat16)

            def seg_reduce(msg, d_t, par, winfo, out_psum, ncols,
                           sc_t=None, transposed=False):
                """selection-matrix segment sum for one window into out_psum.

                transposed: out[feat, dst] via lhsT=msg; else out[dst, feat].
                sc_t: per-slot scale folded into the selection matrix."""
                tot = sum(nr for (_b, _do, _mo, nr) in winfo)
                done = 0
                for (b, do, mo, nr) in winfo:
                    for r in range(nr):
                        # per-partition fp32 scalars keep the DVE 4x mode
                        S = spool.tile([P, P], dt.bfloat16, tag="S")
                        if sc_t is not None:
                            nc.vector.tensor_scalar(
                                S[:], io_t[:, par, :], d_t[:, do + r:do + r + 1],
                                sc_t[:, do + r:do + r + 1],
                                AluOp.is_equal, AluOp.mult)
                        else:
                            nc.vector.tensor_scalar(
                                S[:], io_t[:, par, :], d_t[:, do + r:do + r + 1],
                                None, AluOp.is_equal)
                        if transposed:
                            nc.tensor.matmul(out_psum[:], lhsT=msg[:, mo + r, :ncols],
                                             rhs=S[:], start=(done == 0),
                                             stop=(done == tot - 1))
                        else:
                            nc.tensor.matmul(out_psum[:], lhsT=S[:],
                                             rhs=msg[:, mo + r, :ncols],
                                             start=(done == 0),
                                             stop=(done == tot - 1))
                        done += 1
                return tot

            # ---------------- Layer 1 ----------------
            i2_t = cpool.tile([P, T16_2], dt.int16)
            d2_t = cpool.tile([P, R2], dt.float32)
            for ci, ch in enumerate(l1_chunks):
                msg = mbufs[ci % NMB]
                for (b, i16o, call_len, mo, n_ranks, do) in ch["calls"]:
                    tbl = t_xlo[:] if b == 0 else t_xhi[:]
                    nc.gpsimd.dma_gather(
                        msg[:, mo:mo + n_ranks, :], tbl,
                        i1_t[:, i16o:i16o + call_len // 16],
                        call_len, call_len, DIN, single_packet=False)

                # phase 1: aggregation + h blocks (sels -> amT -> ph -> hT);
                # phase 2: output projections. Splitting keeps the PE queue
                # free of long Act-dependent stalls between windows.
                hts = {}
                for w in ch["ws"]:
                    winfo = ch["windows"][w]
                    amT = wpool.tile([P, P], dt.bfloat16, tag="amT")
                    if winfo:
                        pagg = ppool.tile([P, P], dt.float32, tag="pagg")
                        seg_reduce(msg, d1_t, w & 1, winfo, pagg, DIN,
                                   sc_t=s1_t, transposed=True)
                        nc.scalar.activation(amT[:], pagg[:], AF.Copy)
                    else:
                        nc.vector.memset(amT[:], 0.0)
                    hts[w] = []
                    for blk in range(HID // P):
                        ph = ppoolb.tile([P, P], dt.float32, tag="ph", bufs=2)
                        nc.tensor.matmul(ph[:], lhsT=w1l_t[:, blk * P:(blk + 1) * P],
                                         rhs=amT[:], start=True, stop=False)
                        nc.tensor.matmul(ph[:], lhsT=w1r_t[:, blk * P:(blk + 1) * P],
                                         rhs=xoT_t[:, w * P:(w + 1) * P],
                                         start=False, stop=True)
                        hT = wpool.tile([P, P], dt.bfloat16, tag="hT",
                                        bufs=4 * CW1 + 2)
                        nc.scalar.activation(hT[:], ph[:], AF.Relu,
                                             bias=b1_t[:, blk:blk + 1])
                        hts[w].append(hT)
                for w in ch["ws"]:
                    pq = ppool.tile([P, OUT], dt.float32, tag="pq")
                    qq = ppool.tile([P, OUT], dt.float32, tag="qq")
                    for blk in range(HID // P):
                        hT = hts[w][blk]
                        nc.tensor.matmul(pq[:], lhsT=hT[:], rhs=w2l_t[:, blk, :],
                                         start=(blk == 0), stop=(blk == 3))
                        nc.tensor.matmul(qq[:], lhsT=hT[:], rhs=w2r_t[:, blk, :],
                                         start=(blk == 0), stop=(blk == 3))
                    nc.scalar.activation(qbuf[:, w, :], qq[:], AF.Copy)
                    nc.scalar.activation(pbuf[:, w, :], pq[:], AF.Copy)
                if ci == 0:
                    # tiny DVE writes create a hazard that holds the L2 index
                    # loads (issued at ci==1) off the DMA queue until the DVE
                    # has worked through chunk 0 — keeping startup DMA clear
                    nc.vector.memset(i2_t[:, :16], 0)
                    nc.vector.memset(d2_t[:, :1], 0.0)
                if ci == 1:
                    nc.sync.dma_start(i2_t[:], t_i2[:])
                    nc.sync.dma_start(d2_t[:], t_d2[:])
            # only the 40 real columns travel; the 88 pad columns of each
            # 256B table row are gathered but never read by the matmuls
            nc.sync.dma_start(t_p[:].rearrange("(w p) c -> p w c", p=P)[:, :, :OUT],
                              pbuf[:])

            # qbuf += b2 (once)
            nc.vector.tensor_tensor(out=qbuf[:], in0=qbuf[:],
                                    in1=b2_t[:, None, :].to_broadcast([P, NWIN, OUT]),
                                    op=AluOp.add)

            # ---------------- Layer 2 partials ----------------
            for ci, ch in enumerate(l2_chunks):
                msg = mbufs[ci % NMB]
                for (b, i16o, call_len, mo, n_ranks, do) in ch["calls"]:
                    nc.gpsimd.dma_gather(
                        msg[:, mo:mo + n_ranks, :], t_p[:],
                        i2_t[:, i16o:i16o + call_len // 16],
                        call_len, call_len, P, single_packet=False)
                slab, w0l = ch["w0"] // NWIN, ch["w0"] % NWIN
                nwch = len(ch["ws"])
                pst = wpool.tile([P, CW2, OUT], dt.bfloat16, tag="pst")
                for w in ch["ws"]:
                    winfo = ch["windows"][w]
                    pagg = ppool.tile([P, OUT], dt.float32, tag="pagg")
                    if winfo:
                        seg_reduce(msg, d2_t, w & 1, winfo, pagg, OUT)
                        nc.scalar.activation(pst[:, w - ch["w0"], :], pagg[:], AF.Copy)
                    else:
                        nc.vector.memset(pst[:, w - ch["w0"], :], 0.0)
                nc.sync.dma_start(t_partial[slab, :, w0l:w0l + nwch, :],
                                  pst[:, :nwch, :])

            # ---------------- ReduceScatter + output ----------------
            nc.gpsimd.collective_compute(
                "ReduceScatter", AluOp.add, replica_groups=[list(range(NCORES))],
                ins=[t_partial[:]], outs=[t_rsout[:]])
            nc.sync.dma_start(rs_t[:], t_rsout[:])
            # fully independent per-half chains (DVE/Act/DMA pipeline)
            out_r = t_out[:].rearrange("(w p) o -> p w o", p=P)
            halves = [(0, NWIN // 2), (NWIN // 2, NWIN)]
            for a, b in halves:
                nc.vector.tensor_tensor(
                    out=zbuf[:, a:b, :], in0=rs_t[:, a:b, :],
                    in1=rc_t[:, a:b, None].to_broadcast([P, b - a, OUT]),
                    op=AluOp.mult)
                nc.vector.tensor_tensor(out=zbuf[:, a:b, :], in0=zbuf[:, a:b, :],
                                        in1=qbuf[:, a:b, :], op=AluOp.add)
                nc.scalar.activation(ebuf[:, a:b, :], zbuf[:, a:b, :], AF.Exp)
                nc.vector.tensor_reduce(ssum[:, a:b, :], ebuf[:, a:b, :],
                                        axis=mybir.AxisListType.X, op=AluOp.add)
                nc.scalar.activation(lsum[:, a:b, :], ssum[:, a:b, :], AF.Ln)
                nc.vector.tensor_tensor(
                    out=ebuf[:, a:b, :], in0=zbuf[:, a:b, :],
                    in1=lsum[:, a:b, :].to_broadcast([P, b - a, OUT]),
                    op=AluOp.subtract)
                nc.sync.dma_start(out_r[:, a:b, :], ebuf[:, a:b, :])

    nc.compile()

    in_maps = []
    for c in range(NCORES):
        i1a, d1a, s1a = l1_pc[c]
        i2a, d2a, _ = l2_pc[c]
        xoT = np.zeros((P, NPAD), bf16)
        xoT[:, perm[c]] = x[c * NLOC:(c + 1) * NLOC].T.astype(bf16)
        rcf = np.ones(NPAD, np.float32)
        rcf[perm[c]] = recip[c * NLOC:(c + 1) * NLOC]
        rcc = rcf.reshape(NWIN, P).T.copy()
        in_maps.append({
            "xlo": xlo, "xhi": xhi, "xoT": xoT,
            "i1": i1a, "d1": d1a, "s1": s1a, "i2": i2a, "d2": d2a,
            "w1l": np.asarray(w1_l).astype(bf16), "w1r": np.asarray(w1_r).astype(bf16),
            "w2l": np.ascontiguousarray(np.asarray(w2_l).astype(bf16).reshape(HID // P, P, OUT).transpose(1, 0, 2)),
            "w2r": np.ascontiguousarray(np.asarray(w2_r).astype(bf16).reshape(HID // P, P, OUT).transpose(1, 0, 2)),
            "b1": np.asarray(b1, np.float32).reshape(HID // P, P).T.copy(),
            "b2b": b2b_np, "rc": rcc, "iopar": iopar_np,
        })
    res = run_bass_kernel_spmd(nc, in_maps, list(range(NCORES)))
    out = np.concatenate([res.results[c]["out"][perm[c]] for c in range(NCORES)],
                         axis=0)
    kernel.last_results = res
    kernel.last_nc = nc
    return out.astype(np.float32)


# revision 84
# speedup vs baseline: 2.0460x; 1.0227x over previous
"""GraphSAGE (2-layer, mean-agg) Trainium2 Bass kernel, 8-core SPMD.

Layer 1: dst-sharded (6250 nodes/core, 49 windows of 128). Messages x[src]
fetched with gpsimd dma_gather (bf16 tables split at 32768 so indices fit
int16); segment-sum via per-rank selection-matrix matmuls accumulated in PSUM.
Window slots are 16-granular (not 128) inside each chunk call; chunk-relative
int16 drel + per-window iota tables disambiguate shared boundary ranks.

Layer 2: src-sharded. Each core computes p = relu(h) @ w2_l for its own nodes,
writes it to a private DRAM table, gathers its own-src edges' p rows (single
int16 bucket), and accumulates partial dst sums for all 8 slabs of 49 windows.
One bf16 ReduceScatter (4MB -> 0.5MB) replaces the baseline's 4 AllGathers.

Output: z = rs*recip + (h@w2_r + b2); log_softmax computed with batched Exp
over all windows and a single Ln (avoids activation-table reloads); logits
are within +-5 so no max-subtraction is needed.
"""
import numpy as np
import ml_dtypes

N = 50000
E = 800000
DIN, HID, OUT = 128, 512, 40
NCORES = 8
NLOC = N // NCORES          # 6250
P = 128
NWIN = (NLOC + P - 1) // P  # 49
NPAD = NWIN * P             # 6272
XSPLIT = 32768              # x table split for int16 gather indices
CW1 = 5                     # L1 windows per gather chunk
CW2 = 25                    # L2 windows per gather chunk (within a slab)
HALF1 = 25                  # L2 window split for the two ReduceScatters
KB = 4                      # selection-matrix ranks per DVE build op

bf16 = ml_dtypes.bfloat16


def _cdiv(a, b):
    return -(-a // b)


def _wrap_idx(flat):
    """int16 wrapped layout for one gather call: slot i -> [i%16, i//16]."""
    n = len(flat)
    w = flat.astype(np.int16).reshape(n // 16, 16).T.copy()
    return np.tile(w, (8, 1))  # [128, n/16]


def _chunk_ranges(nwin_total, cw, period, tail):
    """Window ranges of <=cw windows that never cross a period boundary.
    The final range is re-split into ranges of <=tail windows so the
    post-stream compute drain is short."""
    out = []
    for p0 in range(0, nwin_total, period):
        pend = min(p0 + period, nwin_total)
        for w0 in range(p0, pend, cw):
            out.append((w0, min(w0 + cw, pend)))
    a, b = out.pop()
    for w0 in range(a, b, tail):
        out.append((w0, min(w0 + tail, b)))
    return out


def _build_layer(edge_core_lists, nwin_total, cw, nbuckets, period=None,
                 edge_scale=None, tail=None, full_pad_chunks=0,
                 order_key=None):
    """Common (cross-core) schedule for one layer.

    edge_core_lists: per core, (idx, drel_global, win, bucket) arrays where
    win in [0, nwin_total), drel_global = dst offset within the window space
    (win*128 + in-window row). edge_scale: per core, per-edge scale values
    packed alongside drel (selection-matrix entries become this scale).
    Returns (per-core packed (idx16, d, scale) arrays, chunk descriptors,
    totals).
    """
    counts = np.zeros((NCORES, nwin_total, nbuckets), np.int64)
    for c, (gi, dg, win, bk) in enumerate(edge_core_lists):
        np.add.at(counts[c], (win, bk), 1)
    seg16 = counts.max(axis=0).astype(np.int64)  # [nwin, nb] exact max slots
    # a 128-slot rank must never span 3 windows: with >=128-slot segments a
    # rank touches only adjacent windows, which the parity offset in drel
    # disambiguates (values stay < 256 so they are exact in bf16)
    seg16[seg16 > 0] = np.maximum(seg16[seg16 > 0], P)

    chunks = []
    d_off = 0
    i16_off = 0
    ranges = _chunk_ranges(nwin_total, cw, period or nwin_total,
                           tail or max(2, cw // 2))
    if order_key is not None:
        ranges.sort(key=order_key)
    for ci, (w0, wend) in enumerate(ranges):
        ws = list(range(w0, wend))
        calls = []
        windows = {w: [] for w in ws}
        msg_off = 0
        for b in range(nbuckets):
            # first-use chunks of each message buffer gather full ranks so
            # no SBUF slot is ever read uninitialized (pad idx 0, drel -1)
            gran = P if ci < full_pad_chunks else 16
            call_len = _cdiv(int(seg16[ws, b].sum()), gran) * gran
            if call_len == 0:
                continue
            n_ranks = _cdiv(call_len, P)
            off = 0
            for w in ws:
                sl = int(seg16[w, b])
                if sl == 0:
                    continue
                ra, rb = off >> 7, _cdiv(off + sl, P)
                windows[w].append((b, d_off + ra, msg_off + ra, rb - ra))
                off += sl
            calls.append((b, i16_off, call_len, msg_off, n_ranks, d_off))
            d_off += n_ranks
            i16_off += call_len // 16
            msg_off += n_ranks
        chunks.append({
            "w0": w0, "ws": ws, "calls": calls, "windows": windows,
            "n_ranks": msg_off,
        })

    # per-core packed arrays
    per_core = []
    for c, (gi, dg, win, bk) in enumerate(edge_core_lists):
        i16_cols = np.zeros((P, i16_off), np.int16)
        d_cols = np.full((P, d_off), -1, np.float32)
        s_cols = np.zeros((P, d_off), np.float32)
        sv = edge_scale[c] if edge_scale is not None else None
        # bucket edge data sorted by (win, bucket) for slot placement
        order = np.lexsort((bk, win))
        gi_s, dg_s, win_s, bk_s = gi[order], dg[order], win[order], bk[order]
        sv_s = sv[order] if sv is not None else None
        # start index of each (win,bucket) group in sorted arrays
        key = win_s * nbuckets + bk_s
        starts = np.searchsorted(key, np.arange(nwin_total * nbuckets))
        ends = np.searchsorted(key, np.arange(nwin_total * nbuckets), side="right")
        for ch in chunks:
            for (b, i16o, call_len, mo, n_ranks, do) in ch["calls"]:
                flat = np.zeros(n_ranks * P, np.int64)
                drel = np.full(n_ranks * P, -1, np.int64)
                sval = np.zeros(n_ranks * P, np.float32)
                off = 0
                for w in ch["ws"]:
                    sl = int(seg16[w, b])
                    if sl == 0:
                        continue
                    a, e = starts[w * nbuckets + b], ends[w * nbuckets + b]
                    cnt = e - a
                    flat[off:off + cnt] = gi_s[a:e]
                    # window-relative row + parity offset (exact in bf16)
                    drel[off:off + cnt] = (dg_s[a:e] - w * P) + (w & 1) * P
                    if sv_s is not None:
                        sval[off:off + cnt] = sv_s[a:e]
                    off += sl
                i16_cols[:, i16o:i16o + call_len // 16] = _wrap_idx(flat[:call_len])
                d_cols[:, do:do + n_ranks] = drel.reshape(n_ranks, P).T.astype(np.float32)
                s_cols[:, do:do + n_ranks] = sval.reshape(n_ranks, P).T
        per_core.append((i16_cols, d_cols, s_cols))
    return per_core, chunks, i16_off, d_off


def _cdiv_arr(a, b):
    return -(-a // b)


def _balance_perm(src, dst):
    """Per-core permutation of local node rows that balances, for every
    (dst window, src core) pair, the number of incoming edges — shrinking
    the cross-core max() padding in both layers' gather schedules.
    Returns perm[NCORES, NLOC]: original local idx -> permuted row."""
    sc = src // NLOC
    perm = np.zeros((NCORES, NLOC), np.int64)
    for c in range(NCORES):
        lo = c * NLOC
        m = (dst >= lo) & (dst < lo + NLOC)
        # per-node in-degree vector by src core [NLOC, 8]
        vec = np.zeros((NLOC, NCORES), np.int64)
        np.add.at(vec, (dst[m] - lo, sc[m]), 1)
        tot = vec.sum(axis=1)
        order = np.argsort(-tot, kind="stable")
        wsum = np.zeros((NWIN, NCORES), np.int64)
        wcap = np.full(NWIN, P, np.int64)
        wcap[NWIN - 1] = NLOC - (NWIN - 1) * P
        wfill = np.zeros(NWIN, np.int64)
        for node in order:
            cand = np.flatnonzero(wfill < wcap[:len(wfill)])
            costs = (wsum[cand] + vec[node]).max(axis=1)
            w = cand[np.argmin(costs)]
            wsum[w] += vec[node]
            perm[c, node] = w * P + wfill[w]
            wfill[w] += 1
    return perm


def _build_schedule(edge_index):
    src = np.asarray(edge_index[0], dtype=np.int64)
    dst = np.asarray(edge_index[1], dtype=np.int64)
    deg = np.bincount(dst, minlength=N).astype(np.float32)
    recip = 1.0 / np.maximum(deg, 1.0)
    perm = _balance_perm(src, dst)

    # ---- L1: dst-sharded; selection entries carry 1/deg so the PSUM sum
    # is already the mean ----
    l1_lists, l1_scales = [], []
    for c in range(NCORES):
        lo, hi = c * NLOC, (c + 1) * NLOC
        m = (dst >= lo) & (dst < hi)
        s, dg = src[m], perm[c][dst[m] - lo]
        bk = (s >= XSPLIT).astype(np.int64)
        gi = np.where(bk == 1, s - XSPLIT, s)
        l1_lists.append((gi, dg, dg >> 7, bk))
        l1_scales.append(recip[dst[m]])
    l1_pc, l1_chunks, T16_1, R1 = _build_layer(l1_lists, NWIN, CW1, 2,
                                               edge_scale=l1_scales, tail=4,
                                               full_pad_chunks=4)

    # ---- L2: src-sharded, windows = slab*NWIN + within-slab window ----
    l2_lists = []
    for c in range(NCORES):
        lo, hi = c * NLOC, (c + 1) * NLOC
        m = (src >= lo) & (src < hi)
        s, d = perm[c][src[m] - lo], dst[m]
        slab = d // NLOC
        pr = perm[slab, d - slab * NLOC]
        win = slab * NWIN + (pr >> 7)
        dg = win * P + (pr & (P - 1))
        l2_lists.append((s, dg, win, np.zeros(len(s), np.int64)))
    # half-major chunk order: all slabs' windows 0..24 first, so the first
    # ReduceScatter can launch mid-phase and hide under the gather stream
    l2_pc, l2_chunks, T16_2, R2 = _build_layer(
        l2_lists, NWIN * NCORES, CW2, 1, period=NWIN, tail=3,
        order_key=lambda r: ((r[0] % NWIN) >= HALF1, r[0]))

    return (l1_pc, l1_chunks, T16_1, R1, l2_pc, l2_chunks, T16_2, R2, recip,
            perm)


def kernel(x, edge_index, w1_l, b1, w1_r, w2_l, b2, w2_r):
    import concourse.bacc as bacc
    import concourse.mybir as mybir
    import concourse.tile as tile
    from concourse.bass_utils import run_bass_kernel_spmd
    from concourse.library_config import mlp
    from concourse.masks import make_identity

    x = np.asarray(x, np.float32)
    (l1_pc, l1_chunks, T16_1, R1, l2_pc, l2_chunks, T16_2, R2, recip,
     perm) = _build_schedule(np.asarray(edge_index))

    MR1 = max(ch["n_ranks"] for ch in l1_chunks)   # msg tile ranks (L1)
    MR2 = max(ch["n_ranks"] for ch in l2_chunks)
    MR = max(MR1, MR2)

    xlo = np.ascontiguousarray(x[:XSPLIT].astype(bf16))
    xhi = np.ascontiguousarray(x[XSPLIT:].astype(bf16))
    b2b_np = np.tile(np.asarray(b2, np.float32)[None, :], (P, 1))
    iota = np.arange(P, dtype=np.int64)
    iopar_np = np.broadcast_to((iota[None, None, :] + P * np.arange(2)[None, :, None]),
                               (P, 2, P)).astype(bf16).copy()

    nc = bacc.Bacc("TRN2")
    dt = mybir.dt
    t_xlo = nc.declare_dram_parameter("xlo", [XSPLIT, DIN], dt.bfloat16, isOutput=False)
    t_xhi = nc.declare_dram_parameter("xhi", [N - XSPLIT, DIN], dt.bfloat16, isOutput=False)
    t_xoT = nc.declare_dram_parameter("xoT", [P, NPAD], dt.bfloat16, isOutput=False)
    t_i1 = nc.declare_dram_parameter("i1", [P, T16_1], dt.int16, isOutput=False)
    t_d1 = nc.declare_dram_parameter("d1", [P, R1], dt.float32, isOutput=False)
    t_s1 = nc.declare_dram_parameter("s1", [P, R1], dt.float32, isOutput=False)
    t_i2 = nc.declare_dram_parameter("i2", [P, T16_2], dt.int16, isOutput=False)
    t_d2 = nc.declare_dram_parameter("d2", [P, R2], dt.float32, isOutput=False)
    t_w1l = nc.declare_dram_parameter("w1l", [DIN, HID], dt.bfloat16, isOutput=False)
    t_w1r = nc.declare_dram_parameter("w1r", [DIN, HID], dt.bfloat16, isOutput=False)
    t_w2l = nc.declare_dram_parameter("w2l", [P, HID // P, OUT], dt.bfloat16, isOutput=False)
    t_w2r = nc.declare_dram_parameter("w2r", [P, HID // P, OUT], dt.bfloat16, isOutput=False)
    t_b1 = nc.declare_dram_parameter("b1", [P, HID // P], dt.float32, isOutput=False)
    t_b2 = nc.declare_dram_parameter("b2b", [P, OUT], dt.float32, isOutput=False)
    t_rc = nc.declare_dram_parameter("rc", [P, NWIN], dt.float32, isOutput=False)
    t_io = nc.declare_dram_parameter("iopar", [P, 2, P], dt.bfloat16, isOutput=False)
    t_out = nc.declare_dram_parameter("out", [NPAD, OUT], dt.float32, isOutput=True)

    t_p = nc.dram_tensor("ptab", [NPAD, P], dt.bfloat16)
    HB = NWIN - HALF1
    t_pA = nc.dram_tensor("partialA", [NCORES, P, HALF1, OUT], dt.bfloat16)
    t_pB = nc.dram_tensor("partialB", [NCORES, P, HB, OUT], dt.bfloat16)
    t_rsA = nc.dram_tensor("rsoutA", [P, HALF1, OUT], dt.bfloat16)
    t_rsB = nc.dram_tensor("rsoutB", [P, HB, OUT], dt.bfloat16)

    AluOp = mybir.AluOpType
    AF = mybir.ActivationFunctionType

    with tile.TileContext(nc) as tc:
        with tc.tile_pool(name="const", bufs=1) as cpool, \
             tc.tile_pool(name="sm", bufs=6) as spool, \
             tc.tile_pool(name="work", bufs=3) as wpool, \
             tc.tile_pool(name="psumA", bufs=2, space="PSUM") as ppool, \
             tc.tile_pool(name="psumB", bufs=1, space="PSUM") as ppoolb:
            nc.gpsimd.load_library(mlp)
            # chunk 0's slice of i1 first: it alone gates the first gather's
            # descriptor generation; the rest of i1 streams behind it
            c0last = l1_chunks[0]["calls"][-1]
            c0cols = c0last[1] + c0last[2] // 16
            i1_t = cpool.tile([P, T16_1], dt.int16)
            nc.sync.dma_start(i1_t[:, :c0cols], t_i1[:, :c0cols])
            d1_t = cpool.tile([P, R1], dt.float32)
            nc.sync.dma_start(d1_t[:], t_d1[:])
            s1_t = cpool.tile([P, R1], dt.float32)
            nc.sync.dma_start(s1_t[:], t_s1[:])
            io_t = cpool.tile([P, 2, P], dt.bfloat16)
            nc.sync.dma_start(io_t[:], t_io[:])
            nc.sync.dma_start(i1_t[:, c0cols:], t_i1[:, c0cols:])
            xoT_t = cpool.tile([P, NPAD], dt.bfloat16)
            nc.sync.dma_start(xoT_t[:], t_xoT[:])
            b1_t = cpool.tile([P, HID // P], dt.float32)
            nc.sync.dma_start(b1_t[:], t_b1[:])
            b2_t = cpool.tile([P, OUT], dt.float32)
            nc.sync.dma_start(b2_t[:], t_b2[:])
            rc_t = cpool.tile([P, NWIN], dt.float32)
            nc.sync.dma_start(rc_t[:], t_rc[:])
            # message tiles: the first-use gather of buf i covers ranks
            # [0, len_i); only the residue needs zeroing (finite data so
            # S=0 kills any junk in the matmul)
            NMB = 4
            mbufs = [cpool.tile([P, MR, DIN], dt.bfloat16, name=f"mbuf{i}")
                     for i in range(NMB)]
            for i in range(NMB):
                ln = l1_chunks[i]["n_ranks"] if i < len(l1_chunks) else 0
                if ln < MR:
                    nc.vector.memset(mbufs[i][:, ln:, :], 0.0)
            w1l_t = cpool.tile([DIN, HID], dt.bfloat16)
            nc.sync.dma_start(w1l_t[:], t_w1l[:])
            w1r_t = cpool.tile([DIN, HID], dt.bfloat16)
            nc.sync.dma_start(w1r_t[:], t_w1r[:])
            w2l_t = cpool.tile([P, HID // P, OUT], dt.bfloat16)
            nc.sync.dma_start(w2l_t[:], t_w2l[:])
            w2r_t = cpool.tile([P, HID // P, OUT], dt.bfloat16)
            nc.sync.dma_start(w2r_t[:], t_w2r[:])
            qbuf = cpool.tile([P, NWIN, OUT], dt.float32)
            zbuf = cpool.tile([P, NWIN, OUT], dt.float32)
            ebuf = cpool.tile([P, NWIN, OUT], dt.float32)
            ssum = cpool.tile([P, NWIN, 1], dt.float32)
            lsum = cpool.tile([P, NWIN, 1], dt.float32)
            rs_t = cpool.tile([P, NWIN, OUT], dt.bfloat16)
            # p rows accumulate in SBUF; one bulk DRAM write at end of L1
            pbuf = cpool.tile([P, NWIN, OUT], dt.bfloat16)

            def seg_reduce(msg, d_t, par, winfo, out_psum, ncols,
                           sc_t=None, transposed=False):
                """selection-matrix segment sum for one window into out_psum.

                transposed: out[feat, dst] via lhsT=msg; else out[dst, feat].
                sc_t: per-slot scale folded into the selection matrix."""
                tot = sum(nr for (_b, _do, _mo, nr) in winfo)
                done = 0
                for (b, do, mo, nr) in winfo:
                    for r in range(nr):
                        # per-partition fp32 scalars keep the DVE 4x mode
                        S = spool.tile([P, P], dt.bfloat16, tag="S")
                        if sc_t is not None:
                            nc.vector.tensor_scalar(
                                S[:], io_t[:, par, :], d_t[:, do + r:do + r + 1],
                                sc_t[:, do + r:do + r + 1],
                                AluOp.is_equal, AluOp.mult)
                        else:
                            nc.vector.tensor_scalar(
                                S[:], io_t[:, par, :], d_t[:, do + r:do + r + 1],
                                None, AluOp.is_equal)
                        if transposed:
                            nc.tensor.matmul(out_psum[:], lhsT=msg[:, mo + r, :ncols],
                                             rhs=S[:], start=(done == 0),
                                             stop=(done == tot - 1))
                        else:
                            nc.tensor.matmul(out_psum[:], lhsT=S[:],
                                             rhs=msg[:, mo + r, :ncols],
                                             start=(done == 0),
                                             stop=(done == tot - 1))
                        done += 1
                return tot

            # ---------------- Layer 1 ----------------
            i2_t = cpool.tile([P, T16_2], dt.int16)
            d2_t = cpool.tile([P, R2], dt.float32)
            for ci, ch in enumerate(l1_chunks):
                msg = mbufs[ci % NMB]
                for (b, i16o, call_len, mo, n_ranks, do) in ch["calls"]:
                    tbl = t_xlo[:] if b == 0 else t_xhi[:]
                    nc.gpsimd.dma_gather(
                        msg[:, mo:mo + n_ranks, :], tbl,
                        i1_t[:, i16o:i16o + call_len // 16],
                        call_len, call_len, DIN, single_packet=False)

                # phase 1: aggregation + h blocks (sels -> amT -> ph -> hT);
                # phase 2: output projections. Splitting keeps the PE queue
                # free of long Act-dependent stalls between windows.
                hts = {}
                for w in ch["ws"]:
                    winfo = ch["windows"][w]
                    amT = wpool.tile([P, P], dt.bfloat16, tag="amT")
                    if winfo:
                        pagg = ppool.tile([P, P], dt.float32, tag="pagg")
                        seg_reduce(msg, d1_t, w & 1, winfo, pagg, DIN,
                                   sc_t=s1_t, transposed=True)
                        nc.scalar.activation(amT[:], pagg[:], AF.Copy)
                    else:
                        nc.vector.memset(amT[:], 0.0)
                    hts[w] = []
                    for blk in range(HID // P):
                        ph = ppoolb.tile([P, P], dt.float32, tag="ph", bufs=2)
                        nc.tensor.matmul(ph[:], lhsT=w1l_t[:, blk * P:(blk + 1) * P],
                                         rhs=amT[:], start=True, stop=False)
                        nc.tensor.matmul(ph[:], lhsT=w1r_t[:, blk * P:(blk + 1) * P],
                                         rhs=xoT_t[:, w * P:(w + 1) * P],
                                         start=False, stop=True)
                        hT = wpool.tile([P, P], dt.bfloat16, tag="hT",
                                        bufs=4 * CW1 + 2)
                        nc.scalar.activation(hT[:], ph[:], AF.Relu,
                                             bias=b1_t[:, blk:blk + 1])
                        hts[w].append(hT)
                for w in ch["ws"]:
                    pq = ppool.tile([P, OUT], dt.float32, tag="pq")
                    qq = ppool.tile([P, OUT], dt.float32, tag="qq")
                    for blk in range(HID // P):
                        hT = hts[w][blk]
                        nc.tensor.matmul(pq[:], lhsT=hT[:], rhs=w2l_t[:, blk, :],
                                         start=(blk == 0), stop=(blk == 3))
                        nc.tensor.matmul(qq[:], lhsT=hT[:], rhs=w2r_t[:, blk, :],
                                         start=(blk == 0), stop=(blk == 3))
                    nc.scalar.activation(qbuf[:, w, :], qq[:], AF.Copy)
                    nc.scalar.activation(pbuf[:, w, :], pq[:], AF.Copy)
                if ci == 0:
                    # tiny DVE writes create a hazard that holds the L2 index
                    # loads (issued at ci==1) off the DMA queue until the DVE
                    # has worked through chunk 0 — keeping startup DMA clear
                    nc.vector.memset(i2_t[:, :16], 0)
                    nc.vector.memset(d2_t[:, :1], 0.0)
                if ci == 1:
                    nc.sync.dma_start(i2_t[:], t_i2[:])
                    nc.sync.dma_start(d2_t[:], t_d2[:])
            # only the 40 real columns travel; the 88 pad columns of each
            # 256B table row are gathered but never read by the matmuls
            nc.sync.dma_start(t_p[:].rearrange("(w p) c -> p w c", p=P)[:, :, :OUT],
                              pbuf[:])

            # qbuf += b2 (once)
            nc.vector.tensor_tensor(out=qbuf[:], in0=qbuf[:],
                                    in1=b2_t[:, None, :].to_broadcast([P, NWIN, OUT]),
                                    op=AluOp.add)

            # ---------------- Layer 2 partials ----------------
            n_half0 = sum(1 for ch in l2_chunks if (ch["w0"] % NWIN) < HALF1)
            for ci, ch in enumerate(l2_chunks):
                msg = mbufs[ci % NMB]
                for (b, i16o, call_len, mo, n_ranks, do) in ch["calls"]:
                    nc.gpsimd.dma_gather(
                        msg[:, mo:mo + n_ranks, :], t_p[:],
                        i2_t[:, i16o:i16o + call_len // 16],
                        call_len, call_len, P, single_packet=False)
                slab, w0l = ch["w0"] // NWIN, ch["w0"] % NWIN
                nwch = len(ch["ws"])
                pst = wpool.tile([P, CW2, OUT], dt.bfloat16, tag="pst")
                for w in ch["ws"]:
                    winfo = ch["windows"][w]
                    pagg = ppool.tile([P, OUT], dt.float32, tag="pagg")
                    if winfo:
                        seg_reduce(msg, d2_t, w & 1, winfo, pagg, OUT)
                        nc.scalar.activation(pst[:, w - ch["w0"], :], pagg[:], AF.Copy)
                    else:
                        nc.vector.memset(pst[:, w - ch["w0"], :], 0.0)
                if w0l < HALF1:
                    nc.sync.dma_start(t_pA[slab, :, w0l:w0l + nwch, :],
                                      pst[:, :nwch, :])
                else:
                    nc.sync.dma_start(t_pB[slab, :, w0l - HALF1:w0l - HALF1 + nwch, :],
                                      pst[:, :nwch, :])
                if ci == n_half0 - 1:
                    # first-half partials complete on every core: this RS
                    # hides under the second half's gather stream
                    nc.gpsimd.collective_compute(
                        "ReduceScatter", AluOp.add,
                        replica_groups=[list(range(NCORES))],
                        ins=[t_pA[:]], outs=[t_rsA[:]])

            # ---------------- ReduceScatter #2 + output ----------------
            nc.gpsimd.collective_compute(
                "ReduceScatter", AluOp.add, replica_groups=[list(range(NCORES))],
                ins=[t_pB[:]], outs=[t_rsB[:]])
            nc.sync.dma_start(rs_t[:, :HALF1, :], t_rsA[:])
            nc.sync.dma_start(rs_t[:, HALF1:, :], t_rsB[:])
            # per-half chains: half 0 depends only on RS#1, so it overlaps
            # RS#2 on the collective cores
            out_r = t_out[:].rearrange("(w p) o -> p w o", p=P)
            halves = [(0, HALF1), (HALF1, NWIN)]
            for a, b in halves:
                nc.vector.tensor_tensor(
                    out=zbuf[:, a:b, :], in0=rs_t[:, a:b, :],
                    in1=rc_t[:, a:b, None].to_broadcast([P, b - a, OUT]),
                    op=AluOp.mult)
                nc.vector.tensor_tensor(out=zbuf[:, a:b, :], in0=zbuf[:, a:b, :],
                                        in1=qbuf[:, a:b, :], op=AluOp.add)
                nc.scalar.activation(ebuf[:, a:b, :], zbuf[:, a:b, :], AF.Exp)
                nc.vector.tensor_reduce(ssum[:, a:b, :], ebuf[:, a:b, :],
                                        axis=mybir.AxisListType.X, op=AluOp.add)
                nc.scalar.activation(lsum[:, a:b, :], ssum[:, a:b, :], AF.Ln)
                nc.vector.tensor_tensor(
                    out=ebuf[:, a:b, :], in0=zbuf[:, a:b, :],
                    in1=lsum[:, a:b, :].to_broadcast([P, b - a, OUT]),
                    op=AluOp.subtract)
                nc.sync.dma_start(out_r[:, a:b, :], ebuf[:, a:b, :])

    nc.compile()

    in_maps = []
    for c in range(NCORES):
        i1a, d1a, s1a = l1_pc[c]
        i2a, d2a, _ = l2_pc[c]
        xoT = np.zeros((P, NPAD), bf16)
        xoT[:, perm[c]] = x[c * NLOC:(c + 1) * NLOC].T.astype(bf16)
        rcf = np.ones(NPAD, np.float32)
        rcf[perm[c]] = recip[c * NLOC:(c + 1) * NLOC]
        rcc = rcf.reshape(NWIN, P).T.copy()
        in_maps.append({
            "xlo": xlo, "xhi": xhi, "xoT": xoT,
            "i1": i1a, "d1": d1a, "s1": s1a, "i2": i2a, "d2": d2a,
            "w1l": np.asarray(w1_l).astype(bf16), "w1r": np.asarray(w1_r).astype(bf16),
            "w2l": np.ascontiguousarray(np.asarray(w2_l).astype(bf16).reshape(HID // P, P, OUT).transpose(1, 0, 2)),
            "w2r": np.ascontiguousarray(np.asarray(w2_r).astype(bf16).reshape(HID // P, P, OUT).transpose(1, 0, 2)),
            "b1": np.asarray(b1, np.float32).reshape(HID // P, P).T.copy(),
            "b2b": b2b_np, "rc": rcc, "iopar": iopar_np,
        })
    res = run_bass_kernel_spmd(nc, in_maps, list(range(NCORES)))
    out = np.concatenate([res.results[c]["out"][perm[c]] for c in range(NCORES)],
                         axis=0)
    kernel.last_results = res
    kernel.last_nc = nc
    return out.astype(np.float32)
